# revision 1
# baseline (speedup 1.0000x reference)
"""Trainium2 Bass kernel for nn_MultiHeadAttention_36223754174786.

Fused transformer block: QKV projection -> 16-head attention (naive, full
[S,S] scores) -> LayerNorm -> FeedForward (relu MLP) with residual.
B=2, S=2048, D=1024, H=16, DK=64, FF_HIDDEN=2048.

Sharding: data-parallel over tokens across 8 NeuronCores.  Core c handles 512
query tokens of batch b=c//4.  K/V projections for the full batch are
recomputed on each core (replicated inside the 4-core batch group): at these
sizes recompute on the 78 TFLOP/s PE beats moving 16 MB through ~62 GB/s
collectives, so no cross-core communication at all.

Numerics: fp32r (1-pass FP22 multiply) matmuls everywhere except the K/V
projection inputs and FFN weights, which are bf16 (halves their DMA/SBUF).
Attention is a weighted average of V and LayerNorm renormalizes scale, so
*relative* operand error passes straight to the output: bf16 (~0.4%) lands
at ~2.8e-3 of output absmax; fp8 (~4%) was measured at 4e-2 and rejected.

Structure (single TileContext, phases overlap via shared pools):
  p1   qT[D,512] = Wq.T @ xqT (activations arrive host-transposed, so no
       on-device transposes are needed anywhere before attention)
  p3   v token-major [keys, 16 heads, 65] with a ones column per head
  pa   fused K-projection + attention per head pair p (kT rows 128p..):
       kT pair tile (SBUF only) -> transposed scores sT[keys,q] (K=dk=64;
       head pairs auto row-tile via base_partition 0/64) -> exp straight out
       of PSUM on ScalarE (max-subtraction skipped; scores are O(0.4)) ->
       attnT[65,512] = [V_h | 1].T @ expT with the softmax denominator
       accumulating in row 64 -> PE-transpose + per-partition normalize into
       token-major attn.  PE-bound (~183us busy); exp (~135us ACT) hides.
  ln   bn_stats/bn_aggr LayerNorm over the free dim; ln_g==1/ln_b==0 and
       zero biases are specialized away at build time (runtime-checked)
  tr   ffi -> ffiT PE-transpose (FFN contraction needs D on partitions)
  ffn  hT = relu(W1.T @ ffiT) interleaved with ff = hT.T @ W2 first half,
       residual add in token-major, output halves DMA'd as they finish.

Perf (Tile cost model, per core): ~355 us vs ~296 us PE busy-floor.
"""

import numpy as np

import concourse.bass as bass
import concourse.tile as tile
from concourse import bacc, mybir
from concourse.bass_utils import run_bass_kernel_spmd
from concourse.masks import make_identity

F32 = mybir.dt.float32
F32R = mybir.dt.float32r
BF16 = mybir.dt.bfloat16
FP8 = mybir.dt.float8e4
DR = mybir.MatmulPerfMode.DoubleRow
W8SCALE = 64.0
AF = mybir.ActivationFunctionType
OP = mybir.AluOpType

B, S, D, H = 2, 2048, 1024, 16
DK = D // H          # 64
FF = 2048
P = 128
T = 512              # query tokens per core
N_CORES = 8
KC = S // P          # 16 key chunks
QS = T // P          # 4 query sub-tiles
DCH = D // P         # 8 chunks of the model dim
FFC = FF // P        # 16 chunks of the ffn hidden dim
ALL_PHASES = ("p1", "p3", "pa", "ln", "tr", "ffn")


def _bcast_ap(ap):
    """Partition-broadcast a 1-D DRAM vector to [128, n] for DMA."""
    return bass.AP(tensor=ap.tensor, offset=ap.offset, ap=[[0, P]] + list(ap.ap))


def build_program(phases=ALL_PHASES, ln_affine=True, b2_zero=False):
    phases = set(phases)
    nc = bacc.Bacc("TRN2", target_bir_lowering=False, debug=False,
                   num_devices=N_CORES)

    def mm(out_ap, lhsT, rhs, start, stop, perf_mode=None):
        nc.tensor.matmul(out_ap, lhsT, rhs, start=start, stop=stop,
                         perf_mode=perf_mode)

    xqT = nc.dram_tensor("xqT", [D, T], F32R, kind="ExternalInput")
    xkT = nc.dram_tensor("xkT", [D, S], BF16, kind="ExternalInput")
    xvT = nc.dram_tensor("xvT", [D, S], BF16, kind="ExternalInput")
    wq = nc.dram_tensor("wq", [D, D], F32R, kind="ExternalInput")
    wk = nc.dram_tensor("wk", [D, D], BF16, kind="ExternalInput")
    wv = nc.dram_tensor("wv", [D, D], BF16, kind="ExternalInput")
    w1 = nc.dram_tensor("w1", [D, FF], BF16, kind="ExternalInput")
    w2 = nc.dram_tensor("w2", [FF, D], BF16, kind="ExternalInput")
    bq = nc.dram_tensor("bq", [D], F32, kind="ExternalInput")
    bk = nc.dram_tensor("bk", [D], F32, kind="ExternalInput")
    bv = nc.dram_tensor("bv", [D], F32, kind="ExternalInput")
    b1 = nc.dram_tensor("b1", [FF], F32, kind="ExternalInput")
    b2 = nc.dram_tensor("b2", [D], F32, kind="ExternalInput")
    ln_g = nc.dram_tensor("ln_g", [D], F32, kind="ExternalInput")
    ln_b = nc.dram_tensor("ln_b", [D], F32, kind="ExternalInput")
    out = nc.dram_tensor("out", [T, D], F32, kind="ExternalOutput")

    def emit_p1(qT, bq_col, p1w, acc, load_consts):
        xq_sb, wq_sb = [], []
        for k in range(DCH):
            t_ = p1w.tile([P, T], F32R, tag=f"xq{k}", name=f"xq{k}")
            nc.sync.dma_start(t_, xqT[k * P:(k + 1) * P, :])
            xq_sb.append(t_)
            t_ = p1w.tile([P, D], F32R, tag=f"wq{k}", name=f"wq{k}")
            nc.sync.dma_start(t_, wq[k * P:(k + 1) * P, :])
            wq_sb.append(t_)
        load_consts()
        for m in range(DCH):
            ps = acc.tile([P, 512], F32, tag="acc", name="acc")
            for k in range(DCH):
                mm(ps, wq_sb[k][:, m * P:(m + 1) * P], xq_sb[k],
                   start=(k == 0), stop=(k == DCH - 1))
            nc.vector.tensor_scalar_add(qT[m], ps, bq_col[:, m:m + 1])

    def emit_p3(v_sb, bv_b, ones_t, p3w, p3x, acc, prefetch=None):
        wv_sb = []
        for k in range(DCH):
            t_ = p3w.tile([P, D], BF16, tag=f"wv{k}", name=f"wv{k}")
            nc.sync.dma_start(t_, wv[k * P:(k + 1) * P, :])
            wv_sb.append(t_)
        for tg in range(KC // 4):
            xv_t = []
            for k in range(DCH):
                x_ = p3x.tile([P, 512], BF16, tag="xv", name="xv")
                nc.sync.dma_start(
                    x_, xvT[k * P:(k + 1) * P, tg * 512:(tg + 1) * 512])
                xv_t.append(x_)
            if tg == 1 and prefetch is not None:
                nc._xkwk = prefetch()
            for ti in range(4):
                t = tg * 4 + ti
                nc.vector.tensor_copy(v_sb[t][:, :, DK:DK + 1], ones_t)
                for dch in range(2):
                    ps = acc.tile([P, 512], F32, tag="acc", name="acc")
                    for k in range(DCH):
                        mm(ps, xv_t[k][:, ti * P:(ti + 1) * P],
                           wv_sb[k][:, dch * 512:(dch + 1) * 512],
                           start=(k == 0), stop=(k == DCH - 1))
                    nc.vector.tensor_tensor(
                        v_sb[t][:, dch * 8:(dch + 1) * 8, 0:DK],
                        ps[:].rearrange("p (h d) -> p h d", h=8),
                        bv_b[:, dch * 512:(dch + 1) * 512].rearrange(
                            "p (h d) -> p h d", h=8),
                        OP.add)

    def load_xk_wk(p2w):
        xk_sb, wk_sb = [], []
        for k in range(DCH):
            t_ = p2w.tile([P, S], BF16, tag=f"xk{k}", name=f"xk{k}")
            nc.sync.dma_start(t_, xkT[k * P:(k + 1) * P, :])
            xk_sb.append(t_)
            t_ = p2w.tile([P, D], BF16, tag=f"wk{k}", name=f"wk{k}")
            nc.sync.dma_start(t_, wk[k * P:(k + 1) * P, :])
            wk_sb.append(t_)
        return xk_sb, wk_sb

    def emit_p2_attn(qT, v_sb, attn, bk_col, ident, xk_sb, wk_sb,
                     aK, aE, aT, aR, acc, psS, psA, psT):
        """kT head-pair tiles produced in SBUF, consumed immediately by
        scores/exp/attnV/transpose.  One pair = rows 128p..128p+128 of kT."""
        for p in range(H // 2):
            kp = aK.tile([P, S], F32R, tag="kp", name="kp")
            for nch in range(S // 512):
                ps = acc.tile([P, 512], F32, tag="acc", name="acc")
                for k in range(DCH):
                    mm(ps, wk_sb[k][:, p * P:(p + 1) * P],
                       xk_sb[k][:, nch * 512:(nch + 1) * 512],
                       start=(k == 0), stop=(k == DCH - 1))
                nc.vector.tensor_scalar_add(
                    kp[:, nch * 512:(nch + 1) * 512], ps, bk_col[:, p:p + 1])
            for hp in range(2):
                h = 2 * p + hp
                lo, hi = hp * DK, (hp + 1) * DK
                exps = []
                for g in range(8):
                    ps = psS.tile([P, 1024], F32, tag="psS", name="psS")
                    for j in range(2):
                        m = 2 * g + j
                        mm(ps[:, j * 512:(j + 1) * 512],
                           kp[lo:hi, m * P:(m + 1) * P],
                           qT[p][lo:hi, :], start=True, stop=True)
                    e = aE.tile([P, 1024], F32R, tag="exp", name="exp")
                    nc.scalar.activation(e, ps, AF.Exp)
                    exps.append(e)
                pa = psA.tile([P, 512], F32, tag="pa", name="pa")
                for m in range(KC):
                    mm(pa[:DK + 1], v_sb[m][:, h, :],
                       exps[m // 2][:, (m % 2) * 512:(m % 2 + 1) * 512],
                       start=(m == 0), stop=(m == KC - 1))
                at = aT.tile([P, 512], F32, tag="at", name="at")
                nc.vector.tensor_copy(at[:DK + 1], pa[:DK + 1])
                rc = aR.tile([P, QS], F32, tag="rc", name="rc")
                for q in range(QS):
                    pt = psT.tile([P, 512], F32, tag="pa", name="pa")
                    nc.tensor.transpose(
                        pt[:, :DK + 1], at[:DK + 1, q * P:(q + 1) * P],
                        ident[:DK + 1, :DK + 1])
                    nc.vector.reciprocal(rc[:, q:q + 1], pt[:, DK:DK + 1])
                    nc.vector.tensor_scalar_mul(
                        attn[q][:, h * DK:(h + 1) * DK],
                        pt[:, 0:DK], rc[:, q:q + 1])

    def emit_ln_tr(attn, ffi, ffiT, eps_t, lng_b, lnb_b, ident, lnp, psTr):
        for q in range(QS):
            stats = lnp.tile([P, 2, 6], F32, tag="stats", name="stats")
            for sg in range(2):
                nc.vector.bn_stats(stats[:, sg, :],
                                   attn[q][:, sg * 512:(sg + 1) * 512])
            mv = lnp.tile([P, 2], F32, tag="mv", name="mv")
            nc.vector.bn_aggr(mv, stats)
            std = lnp.tile([P, 1], F32, tag="std", name="std")
            nc.scalar.activation(std, mv[:, 1:2], AF.Sqrt, bias=eps_t)
            rstd = lnp.tile([P, 1], F32, tag="rstd", name="rstd")
            nc.vector.reciprocal(rstd, std)
            nc.vector.tensor_scalar(ffi[q], attn[q], mv[:, 0:1], rstd,
                                    OP.subtract, OP.mult)
            if ln_affine:
                nc.vector.tensor_mul(ffi[q], ffi[q], lng_b)
                nc.vector.tensor_add(ffi[q], ffi[q], lnb_b)
            for k in range(DCH):
                pt = psTr.tile([P, P], F32, tag="ptr", name="ptr")
                nc.tensor.transpose(pt, ffi[q][:, k * P:(k + 1) * P], ident)
                nc.vector.tensor_copy(ffiT[k][:, q * P:(q + 1) * P], pt)

    def emit_ffn(ffi, ffiT, out_sb, b1_col, b2_b,
                 hp_, fw1, fw2, psH, psF, out_dma=None):
        hT = [hp_.tile([P, T], BF16, tag=f"hT{f}", name=f"hT{f}")
              for f in range(FFC)]
        w1_sb = []
        for k in range(DCH):
            w1t = fw1.tile([P, FF], BF16, tag=f"w1_{k}", name=f"w1_{k}")
            nc.sync.dma_start(w1t, w1[k * P:(k + 1) * P, :])
            w1_sb.append(w1t)
        pss0 = [psF.tile([P, 512], F32, tag="psF", name="psF")
                for _ in range(QS)]
        for fk in range(FFC):
            ps = psH.tile([P, T], F32, tag="psH", name="psH")
            for k in range(DCH):
                mm(ps, w1_sb[k][:, fk * P:(fk + 1) * P], ffiT[k],
                   start=(k == 0), stop=(k == DCH - 1))
            nc.vector.tensor_scalar(hT[fk], ps, b1_col[:, fk:fk + 1], 0.0,
                                    OP.add, OP.max)
            w2t = fw2.tile([P, 512], BF16, tag="w2a", name="w2a")
            nc.sync.dma_start(w2t, w2[fk * P:(fk + 1) * P, 0:512])
            for q in range(QS):
                mm(pss0[q], hT[fk][:, q * P:(q + 1) * P], w2t,
                   start=(fk == 0), stop=(fk == FFC - 1))
        for q in range(QS):
            nc.vector.tensor_add(out_sb[q][:, 0:512], pss0[q],
                                 ffi[q][:, 0:512])
            if not b2_zero:
                nc.vector.tensor_add(out_sb[q][:, 0:512],
                                     out_sb[q][:, 0:512], b2_b[:, 0:512])
            if out_dma is not None:
                out_dma(q, 0)
        pss1 = [psF.tile([P, 512], F32, tag="psF", name="psF")
                for _ in range(QS)]
        for fk in range(FFC):
            w2t = fw2.tile([P, 512], BF16, tag="w2b", name="w2b")
            nc.sync.dma_start(w2t, w2[fk * P:(fk + 1) * P, 512:1024])
            for q in range(QS):
                mm(pss1[q], hT[fk][:, q * P:(q + 1) * P], w2t,
                   start=(fk == 0), stop=(fk == FFC - 1))
        for q in range(QS):
            nc.vector.tensor_add(out_sb[q][:, 512:1024], pss1[q],
                                 ffi[q][:, 512:1024])
            if not b2_zero:
                nc.vector.tensor_add(out_sb[q][:, 512:1024],
                                     out_sb[q][:, 512:1024],
                                     b2_b[:, 512:1024])
            if out_dma is not None:
                out_dma(q, 1)

    with tile.TileContext(nc) as tc:
        with (
            tc.tile_pool(name="const", bufs=1) as cp,
            tc.tile_pool(name="qTp", bufs=1) as qp,
            tc.tile_pool(name="attnp", bufs=1) as ap_,
            tc.tile_pool(name="accp", bufs=2, space="PSUM") as acc,
        ):
            ident = cp.tile([P, P], F32, tag="ident", name="ident")
            make_identity(nc, ident)
            eps_t = cp.tile([P, 1], F32, tag="eps", name="eps")
            nc.vector.memset(eps_t, 1e-5)
            ones_t = cp.tile([P, H, 1], F32, tag="ones", name="ones")
            nc.vector.memset(ones_t, 1.0)
            bq_col = cp.tile([P, DCH], F32, tag="bqc", name="bqc")
            bk_col = cp.tile([P, DCH], F32, tag="bkc", name="bkc")
            b1_col = cp.tile([P, FFC], F32, tag="b1c", name="b1c")
            lng_b = cp.tile([P, D], F32, tag="lng", name="lng")
            lnb_b = cp.tile([P, D], F32, tag="lnb", name="lnb")
            bv_b = cp.tile([P, D], F32, tag="bvb", name="bvb")
            b2_b = cp.tile([P, D], F32, tag="b2b", name="b2b")

            def load_consts():
                nc.sync.dma_start(bq_col, bq[:].rearrange("(o p) -> p o", p=P))
                nc.sync.dma_start(bk_col, bk[:].rearrange("(o p) -> p o", p=P))
                nc.sync.dma_start(b1_col, b1[:].rearrange("(o p) -> p o", p=P))
                nc.gpsimd.dma_start(lng_b, _bcast_ap(ln_g[:]))
                nc.gpsimd.dma_start(lnb_b, _bcast_ap(ln_b[:]))
                nc.gpsimd.dma_start(bv_b, _bcast_ap(bv[:]))
                nc.gpsimd.dma_start(b2_b, _bcast_ap(b2[:]))

            qT = [qp.tile([P, T], F32R, tag=f"qT{m}", name=f"qT{m}")
                  for m in range(DCH)]
            attn = [ap_.tile([P, D], F32, tag=f"attn{q}", name=f"attn{q}")
                    for q in range(QS)]

            if "p1" in phases:
                with tc.tile_pool(name="p1w", bufs=1) as p1w:
                    emit_p1(qT, bq_col, p1w, acc, load_consts)
            else:
                load_consts()

            with tc.tile_pool(name="vp", bufs=1) as vp:
                v_sb = [vp.tile([P, H, DK + 1], F32R, tag=f"v{t}", name=f"v{t}")
                        for t in range(KC)]
                with tc.tile_pool(name="p2w", bufs=1) as p2w:
                    if "p3" in phases:
                        with (
                            tc.tile_pool(name="p3w", bufs=1) as p3w,
                            tc.tile_pool(name="p3x", bufs=10) as p3x,
                        ):
                            emit_p3(v_sb, bv_b, ones_t, p3w, p3x, acc,
                                    prefetch=(lambda: load_xk_wk(p2w))
                                    if "pa" in phases else None)
                            xk_sb, wk_sb = getattr(nc, "_xkwk", ([], []))
                    elif "pa" in phases:
                        xk_sb, wk_sb = load_xk_wk(p2w)

                    if "pa" in phases:
                        with (
                            tc.tile_pool(name="aK", bufs=2) as aK,
                            tc.tile_pool(name="aE", bufs=6) as aE,
                            tc.tile_pool(name="aT", bufs=2) as aT,
                            tc.tile_pool(name="aR", bufs=2) as aR,
                            tc.tile_pool(name="psS", bufs=2, space="PSUM") as psS,
                            tc.tile_pool(name="psA", bufs=2, space="PSUM") as psA,
                        ):
                            emit_p2_attn(qT, v_sb, attn, bk_col, ident,
                                         xk_sb, wk_sb,
                                         aK, aE, aT, aR, acc, psS, psA, psA)
                            # prewarm the Sqrt ACT table set so the switch
                            # isn't on the LayerNorm critical path
                            warm = aR.tile([P, 1], F32, tag="warm",
                                           name="warm")
                            nc.scalar.activation(warm, eps_t, AF.Sqrt)

            with (
                tc.tile_pool(name="ffip", bufs=1) as fip,
                tc.tile_pool(name="ffiTp", bufs=1) as ftp,
                tc.tile_pool(name="outp", bufs=1) as op_,
            ):
                ffi = [fip.tile([P, D], F32, tag=f"ffi{q}", name=f"ffi{q}")
                       for q in range(QS)]
                ffiT = [ftp.tile([P, T], BF16, tag=f"ffiT{k}", name=f"ffiT{k}")
                        for k in range(DCH)]
                out_sb = [op_.tile([P, D], F32, tag=f"out{q}", name=f"out{q}")
                          for q in range(QS)]

                if "ln" in phases and "tr" in phases:
                    with (
                        tc.tile_pool(name="lnp", bufs=4) as lnp,
                        tc.tile_pool(name="psTr", bufs=4, space="PSUM") as psTr,
                    ):
                        emit_ln_tr(attn, ffi, ffiT, eps_t, lng_b, lnb_b,
                                   ident, lnp, psTr)

                if "ffn" in phases:
                    with (
                        tc.tile_pool(name="hTp", bufs=1) as hp_,
                        tc.tile_pool(name="fw1", bufs=1) as fw1,
                        tc.tile_pool(name="fw2", bufs=4) as fw2,
                        tc.tile_pool(name="psH", bufs=2, space="PSUM") as psH,
                        tc.tile_pool(name="psF", bufs=4, space="PSUM") as psF,
                    ):
                        def out_dma(q, half):
                            sl = slice(half * 512, (half + 1) * 512)
                            nc.sync.dma_start(out[q * P:(q + 1) * P, sl],
                                              out_sb[q][:, sl])
                        emit_ffn(ffi, ffiT, out_sb, b1_col, b2_b,
                                 hp_, fw1, fw2, psH, psF, out_dma=out_dma)

    nc.compile()
    return nc


def kernel(**inputs) -> np.ndarray:
    import ml_dtypes
    f32 = lambda a: np.asarray(a, dtype=np.float32)
    query, key, value = f32(inputs["query"]), f32(inputs["key"]), f32(inputs["value"])
    scale = 1.0 / np.sqrt(np.float32(DK))
    wq = np.ascontiguousarray(f32(inputs["Wq"]) * scale)
    bq = f32(inputs["bq"]) * scale
    wk = f32(inputs["Wk"]).astype(ml_dtypes.bfloat16)
    bk = f32(inputs["bk"])
    wv = f32(inputs["Wv"]).astype(ml_dtypes.bfloat16)
    bv = f32(inputs["bv"])
    w1 = f32(inputs["W1"]).astype(ml_dtypes.bfloat16)
    b1 = f32(inputs["b1"])
    w2 = f32(inputs["W2"]).astype(ml_dtypes.bfloat16)
    b2 = f32(inputs["b2"])
    ln_g, ln_b = f32(inputs["ln_g"]), f32(inputs["ln_b"])

    ln_affine = not (np.all(ln_g == 1.0) and np.all(ln_b == 0.0))
    nc = build_program(ln_affine=ln_affine, b2_zero=not b2.any())

    shared = dict(wq=wq, wk=wk, wv=wv, w1=w1, w2=w2, bq=bq, bk=bk, bv=bv,
                  b1=b1, b2=b2, ln_g=ln_g, ln_b=ln_b)
    in_maps = []
    for c in range(N_CORES):
        b = c // 4
        t0 = (c % 4) * T
        in_maps.append(dict(
            xqT=np.ascontiguousarray(query[b, t0:t0 + T, :].T),
            xkT=np.ascontiguousarray(key[b].T).astype(ml_dtypes.bfloat16),
            xvT=np.ascontiguousarray(value[b].T).astype(ml_dtypes.bfloat16),
            **shared,
        ))

    res = run_bass_kernel_spmd(nc, in_maps, list(range(N_CORES)))
    out = np.empty((B, S, D), dtype=np.float32)
    for c in range(N_CORES):
        b = c // 4
        t0 = (c % 4) * T
        out[b, t0:t0 + T, :] = res.results[c]["out"]
    return out



# revision 2
# speedup vs baseline: 1.0119x; 1.0119x over previous
"""Trainium2 Bass kernel for nn_MultiHeadAttention_36223754174786 (TP version).

Fused transformer block: QKV projection -> 16-head attention (naive, full
[S,S] scores) -> LayerNorm -> FeedForward (relu MLP) with residual.
B=2, S=2048, D=1024, H=16, DK=64, FF_HIDDEN=2048.

Sharding (tensor-parallel, per the hint): core c handles batch b=c//4 and
heads [4r:4r+4] (r=c%4) for ALL 2048 tokens of its batch.  This removes the
4x-replicated K/V projection of the data-parallel layout.  After attention,
a per-512-token-chunk ReduceScatter(add) inside each 4-core batch group
redistributes head-sharded attention output to token-sharded full-D rows:
each core contributes its 256 feature columns (others zero-masked via a
per-core mask input, so the sum is a concat) and the scatter hands core r
rows [128r:128r+128] of the chunk.  LN + FFN then run token-sharded (512
tokens/core) exactly like the DP baseline.

Numerics: fp32 PSUM everywhere.  Q/K/V projections and both FFN matmuls run
as 3-term hi/lo fp8e4 DoubleRow products (x_hi*w_hi + x_hi*w_lo + x_lo*w_hi;
the dropped lo*lo term is O(0.03%^2)), which lands at bf16-or-better
accuracy while DoubleRow halves both pass count and per-pass cost.  Scores
and attnV stay f32r/bf16 (exps and v in bf16).  The exchange runs in
bf16.  Measured end-to-end rel err 4.2e-3 (gate 2e-2); TimelineSim
288.9us vs the 353.5us DP baseline (PE busy 198us vs 293us).

Phases per core (single TileContext; FFN chunk j is emitted after attention
q-group j+1 so the ReduceScatter latency hides behind attention):
  q/k/v   3-term fp8 DR projections -> qT,kp (bf16, [256 feat, 2048]) and
          v token-major [keys, 4 heads, 65] with a 64.0 column per head
          (v is stored at 64x scale; softmax normalization cancels it)
  attn    per q-group of 512 queries x 4 heads: transposed scores
          (K=dk=64, head pairs row-tile via base_partition 0/64) -> exp on
          ScalarE -> attnT = [V|64].T @ expT -> PE-transpose + normalize
          -> mask-mult into bf16 staging -> DMA -> ReduceScatter
  ln/ffn  per 128-token chunk: bn_stats LayerNorm, PE-transpose to ffiT
          with DVE hi/lo fp8 split, FFN1 (3-term DR), relu+hi/lo split,
          FFN2 (3-term DR), residual add, output DMA.
"""

import numpy as np

import concourse.bass as bass
import concourse.tile as tile
from concourse import bacc, mybir
from concourse.bass_utils import run_bass_kernel_spmd
from concourse.masks import make_identity

F32 = mybir.dt.float32
F32R = mybir.dt.float32r
BF16 = mybir.dt.bfloat16
FP8 = mybir.dt.float8e4
DR = mybir.MatmulPerfMode.DoubleRow
AF = mybir.ActivationFunctionType
OP = mybir.AluOpType

B, S, D, H = 2, 2048, 1024, 16
DK = D // H          # 64
FF = 2048
P = 128
N_CORES = 8
HL = H // 4          # 4 heads per core
FTL = HL * DK        # 256 local feature columns
KC = S // P          # 16 key chunks
QG = 4               # q groups of 512
DCH = D // P         # 8 contraction chunks of the model dim
FKC = FF // P        # 16 chunks of the ffn hidden dim
WS = 64.0            # fp8 weight scale
RG = [[0, 1, 2, 3], [4, 5, 6, 7]]


def _bcast_ap(ap):
    """Partition-broadcast a 1-D DRAM vector to [128, n] for DMA."""
    return bass.AP(tensor=ap.tensor, offset=ap.offset, ap=[[0, P]] + list(ap.ap))


def build_program(ln_affine=True, b1_zero=True, b2_zero=True,
                  qscale=1.0 / np.sqrt(np.float32(DK)) / (WS * WS)):
    nc = bacc.Bacc("TRN2", target_bir_lowering=False, debug=False,
                   num_devices=N_CORES)

    def mm(out_ap, lhsT, rhs, start, stop, perf_mode=None):
        nc.tensor.matmul(out_ap, lhsT, rhs, start=start, stop=stop,
                         perf_mode=perf_mode)

    # chunk-major fp8 activations: [P, DCH, S] with row k = c*128 + p
    xqh = nc.dram_tensor("xqh", [P, DCH, S], FP8, kind="ExternalInput")
    xql = nc.dram_tensor("xql", [P, DCH, S], FP8, kind="ExternalInput")
    xkh = nc.dram_tensor("xkh", [P, DCH, S], FP8, kind="ExternalInput")
    xkl = nc.dram_tensor("xkl", [P, DCH, S], FP8, kind="ExternalInput")
    xvh = nc.dram_tensor("xvh", [P, DCH, S], FP8, kind="ExternalInput")
    xvl = nc.dram_tensor("xvl", [P, DCH, S], FP8, kind="ExternalInput")
    # chunk-major fp8 weights: [P, DCH, 2, cols] dim2 = hi/lo
    wqt = nc.dram_tensor("wqt", [P, DCH, 2, FTL], FP8, kind="ExternalInput")
    wkt = nc.dram_tensor("wkt", [P, DCH, 2, FTL], FP8, kind="ExternalInput")
    wvt = nc.dram_tensor("wvt", [P, DCH, 2, FTL], FP8, kind="ExternalInput")
    w1t = nc.dram_tensor("w1t", [P, DCH, 2, FF], FP8, kind="ExternalInput")
    w2t = nc.dram_tensor("w2t", [P, FKC, 2, D], FP8, kind="ExternalInput")
    # small consts (pre-scaled on host, see kernel())
    bqc = nc.dram_tensor("bqc", [P, 2], F32, kind="ExternalInput")
    bkc = nc.dram_tensor("bkc", [P, 2], F32, kind="ExternalInput")
    bvv = nc.dram_tensor("bvv", [FTL], F32, kind="ExternalInput")
    b1c = nc.dram_tensor("b1c", [P, FKC], F32, kind="ExternalInput")
    b2v = nc.dram_tensor("b2v", [D], F32, kind="ExternalInput")
    lngv = nc.dram_tensor("lngv", [D], F32, kind="ExternalInput")
    lnbv = nc.dram_tensor("lnbv", [D], F32, kind="ExternalInput")
    maskc = nc.dram_tensor("maskc", [P, 4], F32, kind="ExternalInput")
    out = nc.dram_tensor("out", [4 * P, D], F32, kind="ExternalOutput")
    # exchange buffers (bf16): per 512-token chunk
    ein = [nc.dram_tensor(f"ein{j}", [4 * P, D], BF16) for j in range(QG)]
    eout = [nc.dram_tensor(f"eout{j}", [P, D], BF16) for j in range(QG)]

    def load_hilo_chunks(pool, name, src_h, src_l):
        """8 rotating [P, DCH, 512] tiles (hi+lo per 512-token chunk)."""
        xs = []
        for n in range(S // 512):
            th = pool.tile([P, DCH, 512], FP8, tag="xin", name=f"{name}h{n}")
            nc.sync.dma_start(th, src_h[:, :, n * 512:(n + 1) * 512])
            tl = pool.tile([P, DCH, 512], FP8, tag="xin", name=f"{name}l{n}")
            nc.sync.dma_start(tl, src_l[:, :, n * 512:(n + 1) * 512])
            xs.append((th, tl))
        return xs

    def emit_proj_qk(dst, wt_sb, xs, acc, bias_col, qscale):
        """dst: 2 x [P, S] tiles (feature-major).  12 DR matmuls per 512-col
        chunk: (w_hi,x_hi),(w_lo,x_hi),(w_hi,x_lo) over chunk pairs."""
        for nchk in range(S // 512):
            xh_sb, xl_sb = xs[nchk]
            for ft in range(2):
                ps = acc.tile([P, 512], F32, tag="acc", name="acc")
                n = 0
                for c in range(0, DCH, 2):
                    for x_t, wj in ((xh_sb, 0), (xh_sb, 1), (xl_sb, 0)):
                        mm(ps, wt_sb[:, c:c + 2, wj,
                                     ft * P:(ft + 1) * P],
                           x_t[:, c:c + 2, :],
                           start=(n == 0), stop=(n == 3 * DCH // 2 - 1),
                           perf_mode=DR)
                        n += 1
                if qscale is None:
                    nc.vector.tensor_scalar_add(
                        dst[ft][:, nchk * 512:(nchk + 1) * 512], ps,
                        bias_col[:, ft:ft + 1])
                else:
                    nc.vector.tensor_scalar(
                        dst[ft][:, nchk * 512:(nchk + 1) * 512], ps,
                        qscale, bias_col[:, ft:ft + 1], OP.mult, OP.add)

    def emit_proj_v(v_sb, wt_sb, xs, acc, bv_b, ones_t):
        for t in range(KC):
            xh_sb, xl_sb = xs[t // 4]
            tc0 = (t % 4) * P
            ps = acc.tile([P, 512], F32, tag="acc", name="acc")
            n = 0
            for c in range(0, DCH, 2):
                for x_t, wj in ((xh_sb, 0), (xh_sb, 1), (xl_sb, 0)):
                    mm(ps[:, 0:FTL],
                       x_t[:, c:c + 2, tc0:tc0 + P],
                       wt_sb[:, c:c + 2, wj, :],
                       start=(n == 0), stop=(n == 3 * DCH // 2 - 1),
                       perf_mode=DR)
                    n += 1
            nc.vector.tensor_copy(v_sb[t][:, :, DK:DK + 1], ones_t)
            nc.vector.tensor_tensor(
                v_sb[t][:, :, 0:DK],
                ps[:, 0:FTL].rearrange("p (h d) -> p h d", h=HL),
                bv_b[:].rearrange("p (h d) -> p h d", h=HL),
                OP.add)

    with tile.TileContext(nc) as tc:
        with (
            tc.tile_pool(name="const", bufs=1) as cp,
            tc.tile_pool(name="qTp", bufs=1) as qp,
            tc.tile_pool(name="kpp", bufs=1) as kpp,
            tc.tile_pool(name="vp", bufs=1) as vp,
        ):
            ident = cp.tile([P, P], F32, tag="ident", name="ident")
            make_identity(nc, ident)
            eps_t = cp.tile([P, 1], F32, tag="eps", name="eps")
            nc.vector.memset(eps_t, 1e-5)
            ones_t = cp.tile([P, HL, 1], F32, tag="ones", name="ones")
            nc.vector.memset(ones_t, WS)   # 64.0: cancels v's 64x scale
            bq_col = cp.tile([P, 2], F32, tag="bqc", name="bqc")
            bk_col = cp.tile([P, 2], F32, tag="bkc", name="bkc")
            b1_col = cp.tile([P, FKC], F32, tag="b1c", name="b1c")
            bv_b = cp.tile([P, FTL], F32, tag="bvb", name="bvb")
            mask_c = cp.tile([P, 4], F32, tag="mkc", name="mkc")
            # small consts go on the Activation DMA queue so they don't
            # delay the first projection input chunks on the SP queue
            nc.scalar.dma_start(bq_col, bqc[:])
            nc.scalar.dma_start(bk_col, bkc[:])
            nc.scalar.dma_start(b1_col, b1c[:])
            nc.scalar.dma_start(mask_c, maskc[:])
            nc.gpsimd.dma_start(bv_b, _bcast_ap(bvv[:]))
            b2_b = lng_b = lnb_b = None
            if not b2_zero:
                b2_b = cp.tile([P, D], F32, tag="b2b", name="b2b")
                nc.gpsimd.dma_start(b2_b, _bcast_ap(b2v[:]))
            if ln_affine:
                lng_b = cp.tile([P, D], F32, tag="lng", name="lng")
                lnb_b = cp.tile([P, D], F32, tag="lnb", name="lnb")
                nc.gpsimd.dma_start(lng_b, _bcast_ap(lngv[:]))
                nc.gpsimd.dma_start(lnb_b, _bcast_ap(lnbv[:]))

            qT = [qp.tile([P, S], BF16, tag=f"qT{m}", name=f"qT{m}")
                  for m in range(2)]
            kp = [kpp.tile([P, S], BF16, tag=f"kp{m}", name=f"kp{m}")
                  for m in range(2)]
            v_sb = [vp.tile([P, HL, DK + 1], BF16, tag=f"v{t}", name=f"v{t}")
                    for t in range(KC)]

            with tc.tile_pool(name="accp", bufs=2, space="PSUM") as acc:
                with (
                    tc.tile_pool(name="xin", bufs=10) as xip,
                    tc.tile_pool(name="wp", bufs=1) as wp,
                ):
                    wk_sb = wp.tile([P, DCH, 2, FTL], FP8, tag="wk",
                                    name="wk")
                    nc.sync.dma_start(wk_sb, wkt[:])
                    xs = load_hilo_chunks(xip, "xk", xkh, xkl)
                    wq_sb = wp.tile([P, DCH, 2, FTL], FP8, tag="wq",
                                    name="wq")
                    nc.sync.dma_start(wq_sb, wqt[:])
                    emit_proj_qk(kp, wk_sb, xs, acc, bk_col, None)
                    xs = load_hilo_chunks(xip, "xq", xqh, xql)
                    wv_sb = wp.tile([P, DCH, 2, FTL], FP8, tag="wv",
                                    name="wv")
                    nc.sync.dma_start(wv_sb, wvt[:])
                    emit_proj_qk(qT, wq_sb, xs, acc, bq_col, float(qscale))
                    xs_v = load_hilo_chunks(xip, "xv", xvh, xvl)
                    emit_proj_v(v_sb, wv_sb, xs_v, acc, bv_b, ones_t)

            from contextlib import ExitStack
            with ExitStack() as _es:
                pools = {}
                for nm, bufs, space in (
                    ("w1p", 1, None), ("w2p", 1, None), ("aE", 12, None),
                    ("aT", 2, None), ("aR", 2, None), ("attnp", 5, None),
                    ("stp", 2, None), ("ffinp", 2, None), ("ffip", 2, None),
                    ("ffiTp", 1, None), ("hTp", 1, None), ("outp", 2, None),
                    ("lnp", 4, None), ("t1p", 2, None),
                    ("psS", 2, "PSUM"), ("psA", 2, "PSUM"),
                    ("psH", 1, "PSUM"), ("psF", 1, "PSUM"),
                ):
                    kw = dict(name=nm, bufs=bufs)
                    if space:
                        kw["space"] = space
                    pools[nm] = _es.enter_context(tc.tile_pool(**kw))
                w1p, w2p, aE, aT, aR = (pools[k] for k in
                                        ("w1p", "w2p", "aE", "aT", "aR"))
                ap_, stp, fin_, fip, ftp = (pools[k] for k in
                                            ("attnp", "stp", "ffinp",
                                             "ffip", "ffiTp"))
                hp_, op_, lnp = pools["hTp"], pools["outp"], pools["lnp"]
                t1p = pools["t1p"]
                psS, psA, psH, psF = (pools[k] for k in
                                      ("psS", "psA", "psH", "psF"))
                w1_sb = w1p.tile([P, DCH, 2, FF], FP8, tag="w1", name="w1")
                nc.sync.dma_start(w1_sb, w1t[:])
                w2_sb = w2p.tile([P, FKC, 2, D], FP8, tag="w2", name="w2")
                nc.sync.dma_start(w2_sb, w2t[:])
                ffiT_h = ftp.tile([P, DCH, 512], FP8, tag="fTh", name="fTh")
                ffiT_l = ftp.tile([P, DCH, 512], FP8, tag="fTl", name="fTl")
                hT_h = hp_.tile([P, FKC, 512], FP8, tag="hTh", name="hTh")
                hT_l = hp_.tile([P, FKC, 512], FP8, tag="hTl", name="hTl")

                def emit_scores_half(qg, h, exps, half):
                    p, hp = h // 2, h % 2
                    lo, hi = hp * DK, (hp + 1) * DK
                    for g in range(half * 4, half * 4 + 4):
                        ps = psS.tile([P, 1024], F32, tag="psS", name="psS")
                        for j in range(2):
                            m = 2 * g + j
                            mm(ps[:, j * 512:(j + 1) * 512],
                               kp[p][lo:hi, m * P:(m + 1) * P],
                               qT[p][lo:hi, qg * 512:(qg + 1) * 512],
                               start=True, stop=True)
                        e = aE.tile([P, 1024], BF16, tag="exp", name="exp")
                        nc.scalar.activation(e, ps, AF.Exp)
                        exps.append(e)

                def emit_attnv(h, exps, attn_t):
                    pa = psA.tile([P, 512], F32, tag="pa", name="pa")
                    for m in range(KC):
                        mm(pa[:DK + 1], v_sb[m][:, h, :],
                           exps[m // 2][:, (m % 2) * 512:(m % 2 + 1) * 512],
                           start=(m == 0), stop=(m == KC - 1))
                    at = aT.tile([P, 512], F32, tag="at", name="at")
                    nc.vector.tensor_copy(at[:DK + 1], pa[:DK + 1])
                    rc = aR.tile([P, 4], F32, tag="rc", name="rc")
                    for qq in range(4):
                        pt = psA.tile([P, 512], F32, tag="pa", name="pa")
                        nc.tensor.transpose(
                            pt[:, :DK + 1], at[:DK + 1, qq * P:(qq + 1) * P],
                            ident[:DK + 1, :DK + 1])
                        nc.vector.reciprocal(rc[:, qq:qq + 1],
                                             pt[:, DK:DK + 1])
                        nc.vector.tensor_scalar_mul(
                            attn_t[qq][:, h * DK:(h + 1) * DK],
                            pt[:, 0:DK], rc[:, qq:qq + 1])

                def emit_staging_rs(qg, attn_t):
                    for qq in range(4):
                        st = stp.tile([P, D], BF16, tag="st", name="st")
                        for g in range(4):
                            nc.vector.tensor_scalar_mul(
                                st[:, g * FTL:(g + 1) * FTL], attn_t[qq],
                                mask_c[:, g:g + 1])
                        nc.sync.dma_start(
                            ein[qg][qq * P:(qq + 1) * P, :], st)
                    nc.gpsimd.collective_compute(
                        "ReduceScatter", OP.add, replica_groups=RG,
                        ins=[ein[qg][:]], outs=[eout[qg][:]])
                    pending.append(emit_ln_ffn_chunk(qg))

                def emit_attn_all(step):
                    """Flat head-stream across all q-groups: head (qg,h)'s
                    attnV runs between the score halves of the next head,
                    ALSO across qg boundaries, so the PE never waits for
                    the previous q-group's trailing exps."""
                    cur = {}

                    def attn_tiles(qg):
                        if qg not in cur:
                            cur[qg] = [ap_.tile([P, FTL], F32, tag="attn",
                                                name=f"at{qg}_{qq}")
                                       for qq in range(4)]
                        return cur[qg]

                    pend = None

                    def flush():
                        nonlocal pend
                        pqg, ph, pexps = pend
                        emit_attnv(ph, pexps, attn_tiles(pqg))
                        if ph == HL - 1:
                            emit_staging_rs(pqg, cur.pop(pqg))
                        pend = None

                    for qg in range(QG):
                        for h in range(HL):
                            exps = []
                            emit_scores_half(qg, h, exps, 0)
                            if pend is not None:
                                flush()
                            emit_scores_half(qg, h, exps, 1)
                            pend = (qg, h, exps)
                            step()
                    flush()

                def emit_ln_ffn_chunk(j):
                    """Generator: yields between pieces so LN/FFN
                    interleaves with the next q-group's attention."""
                    yield
                    yield
                    ffin = fin_.tile([P, D], BF16, tag="ffin", name="ffin")
                    nc.sync.dma_start(ffin, eout[j][:])
                    stats = lnp.tile([P, 2, 6], F32, tag="st", name="st")
                    for sg in range(2):
                        nc.vector.bn_stats(stats[:, sg, :],
                                           ffin[:, sg * 512:(sg + 1) * 512])
                    mv = lnp.tile([P, 2], F32, tag="mv", name="mv")
                    nc.vector.bn_aggr(mv, stats)
                    # rstd = exp(-0.5*ln(var+eps)): Ln and Exp live in the
                    # same ACT table set as the attention Exp, so the LN
                    # path forces no act-table switches (Sqrt would).
                    lnv = lnp.tile([P, 1], F32, tag="sd", name="sd")
                    nc.scalar.activation(lnv, mv[:, 1:2], AF.Ln, bias=eps_t)
                    rstd = lnp.tile([P, 1], F32, tag="rs", name="rs")
                    nc.scalar.activation(rstd, lnv, AF.Exp, scale=-0.5)
                    fj = fip.tile([P, D], F32, tag="ffi", name=f"ffi{j}")
                    nc.vector.tensor_scalar(fj, ffin, mv[:, 0:1], rstd,
                                            OP.subtract, OP.mult)
                    if ln_affine:
                        nc.vector.tensor_mul(fj, fj, lng_b)
                        nc.vector.tensor_add(fj, fj, lnb_b)
                    # transpose to [D, tokens] and split hi/lo fp8.
                    # PSUM comes from the FFN2 pool, NOT psA: sharing psA
                    # would couple the attention attnV/transpose chain to
                    # this RS-gated work and stall the PE FIFO.
                    for cg in range(2):
                        pt = psF.tile([P, 512], F32, tag="psF", name="psF")
                        for c4 in range(4):
                            c8 = cg * 4 + c4
                            nc.tensor.transpose(
                                pt[:, c4 * P:(c4 + 1) * P],
                                fj[:, c8 * P:(c8 + 1) * P], ident)
                        hi_ap = ffiT_h[:, cg * 4:(cg + 1) * 4,
                                       j * P:(j + 1) * P]
                        lo_ap = ffiT_l[:, cg * 4:(cg + 1) * 4,
                                       j * P:(j + 1) * P]
                        nc.vector.tensor_copy(hi_ap, pt.rearrange(
                            "p (c t) -> p c t", c=4))
                        nc.vector.tensor_tensor(lo_ap, pt.rearrange(
                            "p (c t) -> p c t", c=4), hi_ap, OP.subtract)
                    yield
                    # FFN1: psH holds 4 fk regions of 128 tokens
                    for fg in range(4):
                        if fg == 2:
                            yield
                        ps = psH.tile([P, 512], F32, tag="psH", name="psH")
                        for f4 in range(4):
                            fk = fg * 4 + f4
                            reg = ps[:, f4 * P:(f4 + 1) * P]
                            n = 0
                            for c in range(0, DCH, 2):
                                for x_t, wj in ((ffiT_h, 0), (ffiT_h, 1),
                                                (ffiT_l, 0)):
                                    mm(reg,
                                       w1_sb[:, c:c + 2, wj,
                                             fk * P:(fk + 1) * P],
                                       x_t[:, c:c + 2, j * P:(j + 1) * P],
                                       start=(n == 0),
                                       stop=(n == 3 * DCH // 2 - 1),
                                       perf_mode=DR)
                                    n += 1
                        t1 = t1p.tile([P, 512], F32R, tag="t1", name="t1")
                        if b1_zero:
                            nc.scalar.activation(t1, ps, AF.Relu)
                            hi_ap = hT_h[:, fg * 4:(fg + 1) * 4,
                                         j * P:(j + 1) * P]
                            lo_ap = hT_l[:, fg * 4:(fg + 1) * 4,
                                         j * P:(j + 1) * P]
                            t1r = t1.rearrange("p (c t) -> p c t", c=4)
                            nc.vector.tensor_copy(hi_ap, t1r)
                            nc.vector.tensor_tensor(lo_ap, t1r, hi_ap,
                                                    OP.subtract)
                        else:
                            for f4 in range(4):
                                fk = fg * 4 + f4
                                sl = slice(f4 * P, (f4 + 1) * P)
                                nc.vector.tensor_scalar(
                                    t1[:, sl], ps[:, sl],
                                    b1_col[:, fk:fk + 1], 0.0,
                                    OP.add, OP.max)
                            t1r = t1.rearrange("p (c t) -> p c t", c=4)
                            hi_ap = hT_h[:, fg * 4:(fg + 1) * 4,
                                         j * P:(j + 1) * P]
                            lo_ap = hT_l[:, fg * 4:(fg + 1) * 4,
                                         j * P:(j + 1) * P]
                            nc.vector.tensor_copy(hi_ap, t1r)
                            nc.vector.tensor_tensor(lo_ap, t1r, hi_ap,
                                                    OP.subtract)
                    yield
                    # FFN2 + residual
                    o_sb = op_.tile([P, D], F32, tag="out", name="out")
                    for half in range(2):
                        if half == 1:
                            yield
                        ps = psF.tile([P, 512], F32, tag="psF", name="psF")
                        n = 0
                        for fk in range(0, FKC, 2):
                            for x_t, wj in ((hT_h, 0), (hT_h, 1),
                                            (hT_l, 0)):
                                mm(ps,
                                   x_t[:, fk:fk + 2, j * P:(j + 1) * P],
                                   w2_sb[:, fk:fk + 2, wj,
                                         half * 512:(half + 1) * 512],
                                   start=(n == 0),
                                   stop=(n == 3 * FKC // 2 - 1),
                                   perf_mode=DR)
                                n += 1
                        sl = slice(half * 512, (half + 1) * 512)
                        nc.vector.tensor_scalar_mul(
                            o_sb[:, sl], ps, 1.0 / (WS * WS))
                        nc.vector.tensor_add(o_sb[:, sl], o_sb[:, sl],
                                             fj[:, sl])
                        if not b2_zero:
                            nc.vector.tensor_add(o_sb[:, sl], o_sb[:, sl],
                                                 b2_b[:, sl])
                        nc.sync.dma_start(out[j * P:(j + 1) * P, sl],
                                          o_sb[:, sl])

                # software pipeline: LN/FFN pieces of chunk j are
                # drained at points inside attention of later q-groups so
                # the ReduceScatter latency never stalls the PE FIFO
                from collections import deque
                pending = deque()

                def step():
                    while pending:
                        try:
                            next(pending[0])
                            return
                        except StopIteration:
                            pending.popleft()

                def drain_front():
                    # exhaust the current front generator (used to force
                    # V-projection completion before the first attnV)
                    if not pending:
                        return
                    g = pending[0]
                    while True:
                        try:
                            next(g)
                        except StopIteration:
                            pending.popleft()
                            return

                emit_attn_all(step)
                while pending:
                    step()

    nc.compile()
    return nc


def _hilo(a):
    import ml_dtypes
    hi = a.astype(ml_dtypes.float8_e4m3)
    lo = (a - hi.astype(np.float32)).astype(ml_dtypes.float8_e4m3)
    return hi, lo


def _chunk_major(a):
    """[D, S] -> [P, DCH, S] with row k = c*128 + p."""
    return np.ascontiguousarray(a.reshape(DCH, P, -1).transpose(1, 0, 2))


def _w_ilv(w):
    """[D, M] scaled weights -> hi/lo interleaved [P, DCH, 2, M] fp8."""
    hi, lo = _hilo(w)
    M = w.shape[1]
    st = np.stack([hi.reshape(DCH, P, M), lo.reshape(DCH, P, M)], axis=2)
    return np.ascontiguousarray(st.transpose(1, 0, 2, 3))


def _w2_ilv(w):
    """[FF, D] scaled weights -> hi/lo interleaved [P, FKC, 2, D] fp8."""
    hi, lo = _hilo(w)
    st = np.stack([hi.reshape(FKC, P, D), lo.reshape(FKC, P, D)], axis=2)
    return np.ascontiguousarray(st.transpose(1, 0, 2, 3))


def kernel(**inputs) -> np.ndarray:
    f32 = lambda a: np.asarray(a, dtype=np.float32)
    query, key, value = f32(inputs["query"]), f32(inputs["key"]), f32(inputs["value"])
    s = 1.0 / np.sqrt(np.float32(DK))
    Wq, Wk, Wv = f32(inputs["Wq"]), f32(inputs["Wk"]), f32(inputs["Wv"])
    bq, bk, bv = f32(inputs["bq"]), f32(inputs["bk"]), f32(inputs["bv"])
    W1, W2 = f32(inputs["W1"]), f32(inputs["W2"])
    b1, b2 = f32(inputs["b1"]), f32(inputs["b2"])
    ln_g, ln_b = f32(inputs["ln_g"]), f32(inputs["ln_b"])

    ln_affine = not (np.all(ln_g == 1.0) and np.all(ln_b == 0.0))
    nc = build_program(ln_affine=ln_affine, b1_zero=not b1.any(),
                       b2_zero=not b2.any())

    w1i = _w_ilv(W1 * WS)
    w2i = _w2_ilv(W2 * WS)
    b1c = np.ascontiguousarray(b1.reshape(FKC, P).T) * WS
    shared = dict(w1t=w1i, w2t=w2i, b1c=b1c, b2v=b2, lngv=ln_g, lnbv=ln_b)

    in_maps = []
    for c in range(N_CORES):
        b = c // 4
        r = c % 4
        cols = slice(r * FTL, (r + 1) * FTL)
        xqh_, xql_ = _hilo(query[b].T)
        xkh_, xkl_ = _hilo(key[b].T)
        xvh_, xvl_ = _hilo(value[b].T)
        mask = np.zeros((P, 4), np.float32)
        mask[:, r] = 1.0
        in_maps.append(dict(
            xqh=_chunk_major(xqh_), xql=_chunk_major(xql_),
            xkh=_chunk_major(xkh_), xkl=_chunk_major(xkl_),
            xvh=_chunk_major(xvh_), xvl=_chunk_major(xvl_),
            wqt=_w_ilv(Wq[:, cols] * WS),
            wkt=_w_ilv(Wk[:, cols] * WS),
            wvt=_w_ilv(Wv[:, cols] * WS),
            bqc=np.ascontiguousarray(bq[cols].reshape(2, P).T) * (s / WS),
            bkc=np.ascontiguousarray(bk[cols].reshape(2, P).T) * WS,
            bvv=bv[cols] * WS,
            maskc=mask,
            **shared,
        ))

    res = run_bass_kernel_spmd(nc, in_maps, list(range(N_CORES)))
    out = np.empty((B, S, D), dtype=np.float32)
    for c in range(N_CORES):
        b = c // 4
        r = c % 4
        o = res.results[c]["out"]  # [512, D]
        for j in range(QG):
            t0 = j * 512 + r * P
            out[b, t0:t0 + P, :] = o[j * P:(j + 1) * P, :]
    return out


# revision 3
# speedup vs baseline: 1.0219x; 1.0099x over previous
"""Trainium2 Bass kernel for nn_MultiHeadAttention_36223754174786 (TP version).

Fused transformer block: QKV projection -> 16-head attention (naive, full
[S,S] scores) -> LayerNorm -> FeedForward (relu MLP) with residual.
B=2, S=2048, D=1024, H=16, DK=64, FF_HIDDEN=2048.

Sharding (tensor-parallel, per the hint): core c handles batch b=c//4 and
heads [4r:4r+4] (r=c%4) for ALL 2048 tokens of its batch.  This removes the
4x-replicated K/V projection of the data-parallel layout.  After attention,
a per-512-token-chunk ReduceScatter(add) inside each 4-core batch group
redistributes head-sharded attention output to token-sharded full-D rows:
each core contributes its 256 feature columns (others zero-masked via a
per-core mask input, so the sum is a concat) and the scatter hands core r
rows [128r:128r+128] of the chunk.  LN + FFN then run token-sharded (512
tokens/core) exactly like the DP baseline.

Numerics: fp32 PSUM everywhere.  Q/K/V projections and both FFN matmuls run
as 3-term hi/lo fp8e4 DoubleRow products (x_hi*w_hi + x_hi*w_lo + x_lo*w_hi;
the dropped lo*lo term is O(0.03%^2)), which lands at bf16-or-better
accuracy while DoubleRow halves both pass count and per-pass cost.  Scores
and attnV stay f32r/bf16 (exps and v in bf16).  The exchange runs in
bf16.  Measured end-to-end rel err 4.2e-3 (gate 2e-2); TimelineSim
286.0us vs the 353.5us DP baseline (PE busy 198us vs 293us).

Phases per core (single TileContext; FFN chunk j is emitted after attention
q-group j+1 so the ReduceScatter latency hides behind attention):
  q/k/v   3-term fp8 DR projections -> qT,kp (bf16, [256 feat, 2048]) and
          v token-major [keys, 4 heads, 65] with a 64.0 column per head
          (v is stored at 64x scale; softmax normalization cancels it)
  attn    per q-group of 512 queries x 4 heads: transposed scores
          (K=dk=64, head pairs row-tile via base_partition 0/64) -> exp on
          ScalarE -> attnT = [V|64].T @ expT -> PE-transpose + normalize
          -> mask-mult into bf16 staging -> DMA -> ReduceScatter
  ln/ffn  per 128-token chunk: bn_stats LayerNorm, PE-transpose to ffiT
          with DVE hi/lo fp8 split, FFN1 (3-term DR), relu+hi/lo split,
          FFN2 (3-term DR), residual add, output DMA.
"""

import numpy as np

import concourse.bass as bass
import concourse.tile as tile
from concourse import bacc, mybir
from concourse.bass_utils import run_bass_kernel_spmd
from concourse.masks import make_identity

F32 = mybir.dt.float32
F32R = mybir.dt.float32r
BF16 = mybir.dt.bfloat16
FP8 = mybir.dt.float8e4
DR = mybir.MatmulPerfMode.DoubleRow
AF = mybir.ActivationFunctionType
OP = mybir.AluOpType

B, S, D, H = 2, 2048, 1024, 16
DK = D // H          # 64
FF = 2048
P = 128
N_CORES = 8
HL = H // 4          # 4 heads per core
FTL = HL * DK        # 256 local feature columns
KC = S // P          # 16 key chunks
QG = 4               # q groups of 512
DCH = D // P         # 8 contraction chunks of the model dim
FKC = FF // P        # 16 chunks of the ffn hidden dim
WS = 64.0            # fp8 weight scale
RG = [[0, 1, 2, 3], [4, 5, 6, 7]]
NOPS = 2
FGY = 2
STEP2 = False


def _bcast_ap(ap):
    """Partition-broadcast a 1-D DRAM vector to [128, n] for DMA."""
    return bass.AP(tensor=ap.tensor, offset=ap.offset, ap=[[0, P]] + list(ap.ap))


def build_program(ln_affine=True, b1_zero=True, b2_zero=True,
                  qscale=1.0 / np.sqrt(np.float32(DK)) / (WS * WS)):
    nc = bacc.Bacc("TRN2", target_bir_lowering=False, debug=False,
                   num_devices=N_CORES)

    def mm(out_ap, lhsT, rhs, start, stop, perf_mode=None):
        nc.tensor.matmul(out_ap, lhsT, rhs, start=start, stop=stop,
                         perf_mode=perf_mode)

    # chunk-major fp8 activations: [P, DCH, S] with row k = c*128 + p
    xqh = nc.dram_tensor("xqh", [P, DCH, S], FP8, kind="ExternalInput")
    xql = nc.dram_tensor("xql", [P, DCH, S], FP8, kind="ExternalInput")
    xkh = nc.dram_tensor("xkh", [P, DCH, S], FP8, kind="ExternalInput")
    xkl = nc.dram_tensor("xkl", [P, DCH, S], FP8, kind="ExternalInput")
    xvh = nc.dram_tensor("xvh", [P, DCH, S], FP8, kind="ExternalInput")
    xvl = nc.dram_tensor("xvl", [P, DCH, S], FP8, kind="ExternalInput")
    # chunk-major fp8 weights: [P, DCH, 2, cols] dim2 = hi/lo
    wqt = nc.dram_tensor("wqt", [P, DCH, 2, FTL], FP8, kind="ExternalInput")
    wkt = nc.dram_tensor("wkt", [P, DCH, 2, FTL], FP8, kind="ExternalInput")
    wvt = nc.dram_tensor("wvt", [P, DCH, 2, FTL], FP8, kind="ExternalInput")
    w1t = nc.dram_tensor("w1t", [P, DCH, 2, FF], FP8, kind="ExternalInput")
    w2t = nc.dram_tensor("w2t", [P, FKC, 2, D], FP8, kind="ExternalInput")
    # small consts (pre-scaled on host, see kernel())
    bqc = nc.dram_tensor("bqc", [P, 2], F32, kind="ExternalInput")
    bkc = nc.dram_tensor("bkc", [P, 2], F32, kind="ExternalInput")
    bvv = nc.dram_tensor("bvv", [FTL], F32, kind="ExternalInput")
    b1c = nc.dram_tensor("b1c", [P, FKC], F32, kind="ExternalInput")
    b2v = nc.dram_tensor("b2v", [D], F32, kind="ExternalInput")
    lngv = nc.dram_tensor("lngv", [D], F32, kind="ExternalInput")
    lnbv = nc.dram_tensor("lnbv", [D], F32, kind="ExternalInput")
    maskc = nc.dram_tensor("maskc", [P, 4], F32, kind="ExternalInput")
    out = nc.dram_tensor("out", [4 * P, D], F32, kind="ExternalOutput")
    # exchange buffers (bf16): per 512-token chunk
    ein = [nc.dram_tensor(f"ein{j}", [4 * P, D], BF16) for j in range(QG)]
    eout = [nc.dram_tensor(f"eout{j}", [P, D], BF16) for j in range(QG)]

    def load_hilo_chunks(pool, name, src_h, src_l):
        """8 rotating [P, DCH, 512] tiles (hi+lo per 512-token chunk)."""
        xs = []
        for n in range(S // 512):
            th = pool.tile([P, DCH, 512], FP8, tag="xin", name=f"{name}h{n}")
            nc.sync.dma_start(th, src_h[:, :, n * 512:(n + 1) * 512])
            tl = pool.tile([P, DCH, 512], FP8, tag="xin", name=f"{name}l{n}")
            nc.sync.dma_start(tl, src_l[:, :, n * 512:(n + 1) * 512])
            xs.append((th, tl))
        return xs

    def emit_proj_qk(dst, wt_sb, xs, acc, bias_col, qscale):
        """dst: 2 x [P, S] tiles (feature-major).  12 DR matmuls per 512-col
        chunk: (w_hi,x_hi),(w_lo,x_hi),(w_hi,x_lo) over chunk pairs."""
        for nchk in range(S // 512):
            xh_sb, xl_sb = xs[nchk]
            for ft in range(2):
                ps = acc.tile([P, 512], F32, tag="acc", name="acc")
                n = 0
                for c in range(0, DCH, 2):
                    for x_t, wj in ((xh_sb, 0), (xh_sb, 1), (xl_sb, 0)):
                        mm(ps, wt_sb[:, c:c + 2, wj,
                                     ft * P:(ft + 1) * P],
                           x_t[:, c:c + 2, :],
                           start=(n == 0), stop=(n == 3 * DCH // 2 - 1),
                           perf_mode=DR)
                        n += 1
                if qscale is None:
                    nc.vector.tensor_scalar_add(
                        dst[ft][:, nchk * 512:(nchk + 1) * 512], ps,
                        bias_col[:, ft:ft + 1])
                else:
                    nc.vector.tensor_scalar(
                        dst[ft][:, nchk * 512:(nchk + 1) * 512], ps,
                        qscale, bias_col[:, ft:ft + 1], OP.mult, OP.add)

    def emit_proj_v(v_sb, wt_sb, xs, acc, bv_b, ones_t):
        for t in range(KC):
            xh_sb, xl_sb = xs[t // 4]
            tc0 = (t % 4) * P
            ps = acc.tile([P, 512], F32, tag="acc", name="acc")
            n = 0
            for c in range(0, DCH, 2):
                for x_t, wj in ((xh_sb, 0), (xh_sb, 1), (xl_sb, 0)):
                    mm(ps[:, 0:FTL],
                       x_t[:, c:c + 2, tc0:tc0 + P],
                       wt_sb[:, c:c + 2, wj, :],
                       start=(n == 0), stop=(n == 3 * DCH // 2 - 1),
                       perf_mode=DR)
                    n += 1
            nc.vector.tensor_copy(v_sb[t][:, :, DK:DK + 1], ones_t)
            nc.vector.tensor_tensor(
                v_sb[t][:, :, 0:DK],
                ps[:, 0:FTL].rearrange("p (h d) -> p h d", h=HL),
                bv_b[:].rearrange("p (h d) -> p h d", h=HL),
                OP.add)

    with tile.TileContext(nc) as tc:
        with (
            tc.tile_pool(name="const", bufs=1) as cp,
            tc.tile_pool(name="qTp", bufs=1) as qp,
            tc.tile_pool(name="kpp", bufs=1) as kpp,
            tc.tile_pool(name="vp", bufs=1) as vp,
        ):
            ident = cp.tile([P, P], F32, tag="ident", name="ident")
            make_identity(nc, ident)
            eps_t = cp.tile([P, 1], F32, tag="eps", name="eps")
            nc.vector.memset(eps_t, 1e-5)
            ones_t = cp.tile([P, HL, 1], F32, tag="ones", name="ones")
            nc.vector.memset(ones_t, WS)   # 64.0: cancels v's 64x scale
            bq_col = cp.tile([P, 2], F32, tag="bqc", name="bqc")
            bk_col = cp.tile([P, 2], F32, tag="bkc", name="bkc")
            b1_col = cp.tile([P, FKC], F32, tag="b1c", name="b1c")
            bv_b = cp.tile([P, FTL], F32, tag="bvb", name="bvb")
            mask_c = cp.tile([P, 4], F32, tag="mkc", name="mkc")
            # small consts go on the Activation DMA queue so they don't
            # delay the first projection input chunks on the SP queue
            nc.scalar.dma_start(bq_col, bqc[:])
            nc.scalar.dma_start(bk_col, bkc[:])
            nc.scalar.dma_start(b1_col, b1c[:])
            nc.scalar.dma_start(mask_c, maskc[:])
            nc.gpsimd.dma_start(bv_b, _bcast_ap(bvv[:]))
            b2_b = lng_b = lnb_b = None
            if not b2_zero:
                b2_b = cp.tile([P, D], F32, tag="b2b", name="b2b")
                nc.gpsimd.dma_start(b2_b, _bcast_ap(b2v[:]))
            if ln_affine:
                lng_b = cp.tile([P, D], F32, tag="lng", name="lng")
                lnb_b = cp.tile([P, D], F32, tag="lnb", name="lnb")
                nc.gpsimd.dma_start(lng_b, _bcast_ap(lngv[:]))
                nc.gpsimd.dma_start(lnb_b, _bcast_ap(lnbv[:]))

            qT = [qp.tile([P, S], BF16, tag=f"qT{m}", name=f"qT{m}")
                  for m in range(2)]
            kp = [kpp.tile([P, S], BF16, tag=f"kp{m}", name=f"kp{m}")
                  for m in range(2)]
            v_sb = [vp.tile([P, HL, DK + 1], BF16, tag=f"v{t}", name=f"v{t}")
                    for t in range(KC)]

            with tc.tile_pool(name="accp", bufs=2, space="PSUM") as acc:
                with (
                    tc.tile_pool(name="xin", bufs=10) as xip,
                    tc.tile_pool(name="wp", bufs=1) as wp,
                ):
                    wk_sb = wp.tile([P, DCH, 2, FTL], FP8, tag="wk",
                                    name="wk")
                    nc.sync.dma_start(wk_sb, wkt[:])
                    xs = load_hilo_chunks(xip, "xk", xkh, xkl)
                    wq_sb = wp.tile([P, DCH, 2, FTL], FP8, tag="wq",
                                    name="wq")
                    nc.sync.dma_start(wq_sb, wqt[:])
                    emit_proj_qk(kp, wk_sb, xs, acc, bk_col, None)
                    xs = load_hilo_chunks(xip, "xq", xqh, xql)
                    wv_sb = wp.tile([P, DCH, 2, FTL], FP8, tag="wv",
                                    name="wv")
                    nc.sync.dma_start(wv_sb, wvt[:])
                    emit_proj_qk(qT, wq_sb, xs, acc, bq_col, float(qscale))
                    xs_v = load_hilo_chunks(xip, "xv", xvh, xvl)
                    emit_proj_v(v_sb, wv_sb, xs_v, acc, bv_b, ones_t)

            from contextlib import ExitStack
            with ExitStack() as _es:
                pools = {}
                for nm, bufs, space in (
                    ("w1p", 1, None), ("w2p", 1, None), ("aE", 12, None),
                    ("aT", 2, None), ("aR", 2, None), ("attnp", 8, None),
                    ("stp", 4, None), ("ffinp", 2, None), ("ffip", 2, None),
                    ("ffiTp", 1, None), ("hTp", 1, None), ("outp", 2, None),
                    ("lnp", 4, None), ("t1p", 2, None),
                    ("psS", 2, "PSUM"), ("psA", 2, "PSUM"),
                    ("psH", 1, "PSUM"), ("psF", 1, "PSUM"),
                ):
                    kw = dict(name=nm, bufs=bufs)
                    if space:
                        kw["space"] = space
                    pools[nm] = _es.enter_context(tc.tile_pool(**kw))
                w1p, w2p, aE, aT, aR = (pools[k] for k in
                                        ("w1p", "w2p", "aE", "aT", "aR"))
                ap_, stp, fin_, fip, ftp = (pools[k] for k in
                                            ("attnp", "stp", "ffinp",
                                             "ffip", "ffiTp"))
                hp_, op_, lnp = pools["hTp"], pools["outp"], pools["lnp"]
                t1p = pools["t1p"]
                psS, psA, psH, psF = (pools[k] for k in
                                      ("psS", "psA", "psH", "psF"))
                w1_sb = w1p.tile([P, DCH, 2, FF], FP8, tag="w1", name="w1")
                nc.sync.dma_start(w1_sb, w1t[:])
                w2_sb = w2p.tile([P, FKC, 2, D], FP8, tag="w2", name="w2")
                nc.sync.dma_start(w2_sb, w2t[:])
                ffiT_h = ftp.tile([P, DCH, 512], FP8, tag="fTh", name="fTh")
                ffiT_l = ftp.tile([P, DCH, 512], FP8, tag="fTl", name="fTl")
                hT_h = hp_.tile([P, FKC, 512], FP8, tag="hTh", name="hTh")
                hT_l = hp_.tile([P, FKC, 512], FP8, tag="hTl", name="hTl")

                def emit_scores_half(qg, h, exps, half):
                    p, hp = h // 2, h % 2
                    lo, hi = hp * DK, (hp + 1) * DK
                    for g in range(half * 4, half * 4 + 4):
                        ps = psS.tile([P, 1024], F32, tag="psS", name="psS")
                        for j in range(2):
                            m = 2 * g + j
                            mm(ps[:, j * 512:(j + 1) * 512],
                               kp[p][lo:hi, m * P:(m + 1) * P],
                               qT[p][lo:hi, qg * 512:(qg + 1) * 512],
                               start=True, stop=True)
                        e = aE.tile([P, 1024], BF16, tag="exp", name="exp")
                        nc.scalar.activation(e, ps, AF.Exp)
                        exps.append(e)

                def emit_attnv(h, exps, attn_t):
                    pa = psA.tile([P, 512], F32, tag="pa", name="pa")
                    for m in range(KC):
                        mm(pa[:DK + 1], v_sb[m][:, h, :],
                           exps[m // 2][:, (m % 2) * 512:(m % 2 + 1) * 512],
                           start=(m == 0), stop=(m == KC - 1))
                    at = aT.tile([P, 512], F32, tag="at", name="at")
                    nc.vector.tensor_copy(at[:DK + 1], pa[:DK + 1])
                    rc = aR.tile([P, 4], F32, tag="rc", name="rc")
                    for qq in range(4):
                        pt = psA.tile([P, 512], F32, tag="pa", name="pa")
                        nc.tensor.transpose(
                            pt[:, :DK + 1], at[:DK + 1, qq * P:(qq + 1) * P],
                            ident[:DK + 1, :DK + 1])
                        nc.vector.reciprocal(rc[:, qq:qq + 1],
                                             pt[:, DK:DK + 1])
                        nc.vector.tensor_scalar_mul(
                            attn_t[qq][:, h * DK:(h + 1) * DK],
                            pt[:, 0:DK], rc[:, qq:qq + 1])

                def emit_staging_rs(qg, attn_t):
                    for qq in range(4):
                        st = stp.tile([P, D], BF16, tag="st", name="st")
                        for g in range(4):
                            nc.vector.tensor_scalar_mul(
                                st[:, g * FTL:(g + 1) * FTL], attn_t[qq],
                                mask_c[:, g:g + 1])
                        nc.sync.dma_start(
                            ein[qg][qq * P:(qq + 1) * P, :], st)
                    nc.gpsimd.collective_compute(
                        "ReduceScatter", OP.add, replica_groups=RG,
                        ins=[ein[qg][:]], outs=[eout[qg][:]])
                    pending.append(emit_ln_ffn_chunk(qg))

                def emit_attn_all(step):
                    """Flat head-stream across all q-groups: head (qg,h)'s
                    attnV runs between the score halves of the next head,
                    ALSO across qg boundaries, so the PE never waits for
                    the previous q-group's trailing exps."""
                    cur = {}

                    def attn_tiles(qg):
                        if qg not in cur:
                            cur[qg] = [ap_.tile([P, FTL], F32, tag="attn",
                                                name=f"at{qg}_{qq}")
                                       for qq in range(4)]
                        return cur[qg]

                    pend = None

                    def flush():
                        nonlocal pend
                        pqg, ph, pexps = pend
                        emit_attnv(ph, pexps, attn_tiles(pqg))
                        if ph == HL - 1:
                            emit_staging_rs(pqg, cur.pop(pqg))
                        pend = None

                    for qg in range(QG):
                        for h in range(HL):
                            exps = []
                            emit_scores_half(qg, h, exps, 0)
                            if pend is not None:
                                flush()
                            if STEP2:
                                step()
                            emit_scores_half(qg, h, exps, 1)
                            pend = (qg, h, exps)
                            step()
                    flush()

                def emit_ln_ffn_chunk(j):
                    """Generator: yields between pieces so LN/FFN
                    interleaves with the next q-group's attention."""
                    for _ in range(NOPS):
                        yield
                    ffin = fin_.tile([P, D], BF16, tag="ffin", name="ffin")
                    nc.sync.dma_start(ffin, eout[j][:])
                    stats = lnp.tile([P, 2, 6], F32, tag="st", name="st")
                    for sg in range(2):
                        nc.vector.bn_stats(stats[:, sg, :],
                                           ffin[:, sg * 512:(sg + 1) * 512])
                    mv = lnp.tile([P, 2], F32, tag="mv", name="mv")
                    nc.vector.bn_aggr(mv, stats)
                    # rstd = exp(-0.5*ln(var+eps)): Ln and Exp live in the
                    # same ACT table set as the attention Exp, so the LN
                    # path forces no act-table switches (Sqrt would).
                    lnv = lnp.tile([P, 1], F32, tag="sd", name="sd")
                    nc.scalar.activation(lnv, mv[:, 1:2], AF.Ln, bias=eps_t)
                    rstd = lnp.tile([P, 1], F32, tag="rs", name="rs")
                    nc.scalar.activation(rstd, lnv, AF.Exp, scale=-0.5)
                    fj = fip.tile([P, D], F32, tag="ffi", name=f"ffi{j}")
                    nc.vector.tensor_scalar(fj, ffin, mv[:, 0:1], rstd,
                                            OP.subtract, OP.mult)
                    if ln_affine:
                        nc.vector.tensor_mul(fj, fj, lng_b)
                        nc.vector.tensor_add(fj, fj, lnb_b)
                    # transpose to [D, tokens] and split hi/lo fp8.
                    # PSUM comes from the FFN2 pool, NOT psA: sharing psA
                    # would couple the attention attnV/transpose chain to
                    # this RS-gated work and stall the PE FIFO.
                    for cg in range(2):
                        pt = psF.tile([P, 512], F32, tag="psF", name="psF")
                        for c4 in range(4):
                            c8 = cg * 4 + c4
                            nc.tensor.transpose(
                                pt[:, c4 * P:(c4 + 1) * P],
                                fj[:, c8 * P:(c8 + 1) * P], ident)
                        hi_ap = ffiT_h[:, cg * 4:(cg + 1) * 4,
                                       j * P:(j + 1) * P]
                        lo_ap = ffiT_l[:, cg * 4:(cg + 1) * 4,
                                       j * P:(j + 1) * P]
                        nc.vector.tensor_copy(hi_ap, pt.rearrange(
                            "p (c t) -> p c t", c=4))
                        nc.vector.tensor_tensor(lo_ap, pt.rearrange(
                            "p (c t) -> p c t", c=4), hi_ap, OP.subtract)
                    yield
                    # FFN1: psH holds 4 fk regions of 128 tokens
                    for fg in range(4):
                        if fg and fg % FGY == 0:
                            yield
                        ps = psH.tile([P, 512], F32, tag="psH", name="psH")
                        for f4 in range(4):
                            fk = fg * 4 + f4
                            reg = ps[:, f4 * P:(f4 + 1) * P]
                            n = 0
                            for c in range(0, DCH, 2):
                                for x_t, wj in ((ffiT_h, 0), (ffiT_h, 1),
                                                (ffiT_l, 0)):
                                    mm(reg,
                                       w1_sb[:, c:c + 2, wj,
                                             fk * P:(fk + 1) * P],
                                       x_t[:, c:c + 2, j * P:(j + 1) * P],
                                       start=(n == 0),
                                       stop=(n == 3 * DCH // 2 - 1),
                                       perf_mode=DR)
                                    n += 1
                        t1 = t1p.tile([P, 512], F32R, tag="t1", name="t1")
                        if b1_zero:
                            nc.scalar.activation(t1, ps, AF.Relu)
                            hi_ap = hT_h[:, fg * 4:(fg + 1) * 4,
                                         j * P:(j + 1) * P]
                            lo_ap = hT_l[:, fg * 4:(fg + 1) * 4,
                                         j * P:(j + 1) * P]
                            t1r = t1.rearrange("p (c t) -> p c t", c=4)
                            nc.vector.tensor_copy(hi_ap, t1r)
                            nc.vector.tensor_tensor(lo_ap, t1r, hi_ap,
                                                    OP.subtract)
                        else:
                            for f4 in range(4):
                                fk = fg * 4 + f4
                                sl = slice(f4 * P, (f4 + 1) * P)
                                nc.vector.tensor_scalar(
                                    t1[:, sl], ps[:, sl],
                                    b1_col[:, fk:fk + 1], 0.0,
                                    OP.add, OP.max)
                            t1r = t1.rearrange("p (c t) -> p c t", c=4)
                            hi_ap = hT_h[:, fg * 4:(fg + 1) * 4,
                                         j * P:(j + 1) * P]
                            lo_ap = hT_l[:, fg * 4:(fg + 1) * 4,
                                         j * P:(j + 1) * P]
                            nc.vector.tensor_copy(hi_ap, t1r)
                            nc.vector.tensor_tensor(lo_ap, t1r, hi_ap,
                                                    OP.subtract)
                    yield
                    # FFN2 + residual
                    o_sb = op_.tile([P, D], F32, tag="out", name="out")
                    for half in range(2):
                        if half == 1:
                            yield
                        ps = psF.tile([P, 512], F32, tag="psF", name="psF")
                        n = 0
                        for fk in range(0, FKC, 2):
                            for x_t, wj in ((hT_h, 0), (hT_h, 1),
                                            (hT_l, 0)):
                                mm(ps,
                                   x_t[:, fk:fk + 2, j * P:(j + 1) * P],
                                   w2_sb[:, fk:fk + 2, wj,
                                         half * 512:(half + 1) * 512],
                                   start=(n == 0),
                                   stop=(n == 3 * FKC // 2 - 1),
                                   perf_mode=DR)
                                n += 1
                        sl = slice(half * 512, (half + 1) * 512)
                        nc.vector.tensor_scalar_mul(
                            o_sb[:, sl], ps, 1.0 / (WS * WS))
                        nc.vector.tensor_add(o_sb[:, sl], o_sb[:, sl],
                                             fj[:, sl])
                        if not b2_zero:
                            nc.vector.tensor_add(o_sb[:, sl], o_sb[:, sl],
                                                 b2_b[:, sl])
                        nc.sync.dma_start(out[j * P:(j + 1) * P, sl],
                                          o_sb[:, sl])

                # software pipeline: LN/FFN pieces of chunk j are
                # drained at points inside attention of later q-groups so
                # the ReduceScatter latency never stalls the PE FIFO
                from collections import deque
                pending = deque()

                def step():
                    while pending:
                        try:
                            next(pending[0])
                            return
                        except StopIteration:
                            pending.popleft()

                def drain_front():
                    # exhaust the current front generator (used to force
                    # V-projection completion before the first attnV)
                    if not pending:
                        return
                    g = pending[0]
                    while True:
                        try:
                            next(g)
                        except StopIteration:
                            pending.popleft()
                            return

                emit_attn_all(step)
                while pending:
                    step()

    nc.compile()
    return nc


def _hilo(a):
    import ml_dtypes
    hi = a.astype(ml_dtypes.float8_e4m3)
    lo = (a - hi.astype(np.float32)).astype(ml_dtypes.float8_e4m3)
    return hi, lo


def _chunk_major(a):
    """[D, S] -> [P, DCH, S] with row k = c*128 + p."""
    return np.ascontiguousarray(a.reshape(DCH, P, -1).transpose(1, 0, 2))


def _w_ilv(w):
    """[D, M] scaled weights -> hi/lo interleaved [P, DCH, 2, M] fp8."""
    hi, lo = _hilo(w)
    M = w.shape[1]
    st = np.stack([hi.reshape(DCH, P, M), lo.reshape(DCH, P, M)], axis=2)
    return np.ascontiguousarray(st.transpose(1, 0, 2, 3))


def _w2_ilv(w):
    """[FF, D] scaled weights -> hi/lo interleaved [P, FKC, 2, D] fp8."""
    hi, lo = _hilo(w)
    st = np.stack([hi.reshape(FKC, P, D), lo.reshape(FKC, P, D)], axis=2)
    return np.ascontiguousarray(st.transpose(1, 0, 2, 3))


def kernel(**inputs) -> np.ndarray:
    f32 = lambda a: np.asarray(a, dtype=np.float32)
    query, key, value = f32(inputs["query"]), f32(inputs["key"]), f32(inputs["value"])
    s = 1.0 / np.sqrt(np.float32(DK))
    Wq, Wk, Wv = f32(inputs["Wq"]), f32(inputs["Wk"]), f32(inputs["Wv"])
    bq, bk, bv = f32(inputs["bq"]), f32(inputs["bk"]), f32(inputs["bv"])
    W1, W2 = f32(inputs["W1"]), f32(inputs["W2"])
    b1, b2 = f32(inputs["b1"]), f32(inputs["b2"])
    ln_g, ln_b = f32(inputs["ln_g"]), f32(inputs["ln_b"])

    ln_affine = not (np.all(ln_g == 1.0) and np.all(ln_b == 0.0))
    nc = build_program(ln_affine=ln_affine, b1_zero=not b1.any(),
                       b2_zero=not b2.any())

    w1i = _w_ilv(W1 * WS)
    w2i = _w2_ilv(W2 * WS)
    b1c = np.ascontiguousarray(b1.reshape(FKC, P).T) * WS
    shared = dict(w1t=w1i, w2t=w2i, b1c=b1c, b2v=b2, lngv=ln_g, lnbv=ln_b)

    in_maps = []
    for c in range(N_CORES):
        b = c // 4
        r = c % 4
        cols = slice(r * FTL, (r + 1) * FTL)
        xqh_, xql_ = _hilo(query[b].T)
        xkh_, xkl_ = _hilo(key[b].T)
        xvh_, xvl_ = _hilo(value[b].T)
        mask = np.zeros((P, 4), np.float32)
        mask[:, r] = 1.0
        in_maps.append(dict(
            xqh=_chunk_major(xqh_), xql=_chunk_major(xql_),
            xkh=_chunk_major(xkh_), xkl=_chunk_major(xkl_),
            xvh=_chunk_major(xvh_), xvl=_chunk_major(xvl_),
            wqt=_w_ilv(Wq[:, cols] * WS),
            wkt=_w_ilv(Wk[:, cols] * WS),
            wvt=_w_ilv(Wv[:, cols] * WS),
            bqc=np.ascontiguousarray(bq[cols].reshape(2, P).T) * (s / WS),
            bkc=np.ascontiguousarray(bk[cols].reshape(2, P).T) * WS,
            bvv=bv[cols] * WS,
            maskc=mask,
            **shared,
        ))

    res = run_bass_kernel_spmd(nc, in_maps, list(range(N_CORES)))
    out = np.empty((B, S, D), dtype=np.float32)
    for c in range(N_CORES):
        b = c // 4
        r = c % 4
        o = res.results[c]["out"]  # [512, D]
        for j in range(QG):
            t0 = j * 512 + r * P
            out[b, t0:t0 + P, :] = o[j * P:(j + 1) * P, :]
    return out


# revision 4
# speedup vs baseline: 1.0258x; 1.0039x over previous
"""Trainium2 Bass kernel for nn_MultiHeadAttention_36223754174786 (TP version).

Fused transformer block: QKV projection -> 16-head attention (naive, full
[S,S] scores) -> LayerNorm -> FeedForward (relu MLP) with residual.
B=2, S=2048, D=1024, H=16, DK=64, FF_HIDDEN=2048.

Sharding (tensor-parallel, per the hint): core c handles batch b=c//4 and
heads [4r:4r+4] (r=c%4) for ALL 2048 tokens of its batch.  This removes the
4x-replicated K/V projection of the data-parallel layout.  After attention,
a per-512-token-chunk ReduceScatter(add) inside each 4-core batch group
redistributes head-sharded attention output to token-sharded full-D rows:
each core contributes its 256 feature columns (others zero-masked via a
per-core mask input, so the sum is a concat) and the scatter hands core r
rows [128r:128r+128] of the chunk.  LN + FFN then run token-sharded (512
tokens/core) exactly like the DP baseline.

Numerics: fp32 PSUM everywhere.  Q/K/V projections and both FFN matmuls run
as 3-term hi/lo fp8e4 DoubleRow products (x_hi*w_hi + x_hi*w_lo + x_lo*w_hi;
the dropped lo*lo term is O(0.03%^2)), which lands at bf16-or-better
accuracy while DoubleRow halves both pass count and per-pass cost.  Scores
and attnV stay f32r/bf16 (exps and v in bf16).  The exchange runs in
bf16.  Measured end-to-end rel err 4.2e-3 (gate 2e-2); TimelineSim
284.9us vs the 353.5us DP baseline (PE busy 198us vs 293us).

Phases per core (single TileContext; FFN chunk j is emitted after attention
q-group j+1 so the ReduceScatter latency hides behind attention):
  q/k/v   3-term fp8 DR projections -> qT,kp (bf16, [256 feat, 2048]) and
          v token-major [keys, 4 heads, 65] with a 64.0 column per head
          (v is stored at 64x scale; softmax normalization cancels it)
  attn    per q-group of 512 queries x 4 heads: transposed scores
          (K=dk=64, head pairs row-tile via base_partition 0/64) -> exp on
          ScalarE -> attnT = [V|64].T @ expT -> PE-transpose + normalize
          -> mask-mult into bf16 staging -> DMA -> ReduceScatter
  ln/ffn  per 128-token chunk: bn_stats LayerNorm, PE-transpose to ffiT
          with DVE hi/lo fp8 split, FFN1 (3-term DR), relu+hi/lo split,
          FFN2 (3-term DR), residual add, output DMA.
"""

import numpy as np

import concourse.bass as bass
import concourse.tile as tile
from concourse import bacc, mybir
from concourse.bass_utils import run_bass_kernel_spmd
from concourse.masks import make_identity

F32 = mybir.dt.float32
F32R = mybir.dt.float32r
BF16 = mybir.dt.bfloat16
FP8 = mybir.dt.float8e4
DR = mybir.MatmulPerfMode.DoubleRow
AF = mybir.ActivationFunctionType
OP = mybir.AluOpType

B, S, D, H = 2, 2048, 1024, 16
DK = D // H          # 64
FF = 2048
P = 128
N_CORES = 8
HL = H // 4          # 4 heads per core
FTL = HL * DK        # 256 local feature columns
KC = S // P          # 16 key chunks
QG = 4               # q groups of 512
DCH = D // P         # 8 contraction chunks of the model dim
FKC = FF // P        # 16 chunks of the ffn hidden dim
WS = 64.0            # fp8 weight scale
RG = [[0, 1, 2, 3], [4, 5, 6, 7]]
NOPS = 2
FGY = 2
STEP2 = False


def _bcast_ap(ap):
    """Partition-broadcast a 1-D DRAM vector to [128, n] for DMA."""
    return bass.AP(tensor=ap.tensor, offset=ap.offset, ap=[[0, P]] + list(ap.ap))


def build_program(ln_affine=True, b1_zero=True, b2_zero=True,
                  qscale=1.0 / np.sqrt(np.float32(DK)) / (WS * WS)):
    nc = bacc.Bacc("TRN2", target_bir_lowering=False, debug=False,
                   num_devices=N_CORES)

    def mm(out_ap, lhsT, rhs, start, stop, perf_mode=None):
        nc.tensor.matmul(out_ap, lhsT, rhs, start=start, stop=stop,
                         perf_mode=perf_mode)

    # chunk-major fp8 activations: [P, DCH, 2, S], row k = c*128 + p,
    # dim2 = hi/lo (interleaved so each chunk is ONE DMA, halving the
    # HWDGE serialization during the DMA-bound startup)
    xq = nc.dram_tensor("xq", [P, DCH, 2, S], FP8, kind="ExternalInput")
    xk = nc.dram_tensor("xk", [P, DCH, 2, S], FP8, kind="ExternalInput")
    xv = nc.dram_tensor("xv", [P, DCH, 2, S], FP8, kind="ExternalInput")
    # chunk-major fp8 weights: [P, DCH, 2, cols] dim2 = hi/lo
    wqt = nc.dram_tensor("wqt", [P, DCH, 2, FTL], FP8, kind="ExternalInput")
    wkt = nc.dram_tensor("wkt", [P, DCH, 2, FTL], FP8, kind="ExternalInput")
    wvt = nc.dram_tensor("wvt", [P, DCH, 2, FTL], FP8, kind="ExternalInput")
    w1t = nc.dram_tensor("w1t", [P, DCH, 2, FF], FP8, kind="ExternalInput")
    w2t = nc.dram_tensor("w2t", [P, FKC, 2, D], FP8, kind="ExternalInput")
    # small consts (pre-scaled on host, see kernel())
    bqc = nc.dram_tensor("bqc", [P, 2], F32, kind="ExternalInput")
    bkc = nc.dram_tensor("bkc", [P, 2], F32, kind="ExternalInput")
    bvv = nc.dram_tensor("bvv", [FTL], F32, kind="ExternalInput")
    b1c = nc.dram_tensor("b1c", [P, FKC], F32, kind="ExternalInput")
    b2v = nc.dram_tensor("b2v", [D], F32, kind="ExternalInput")
    lngv = nc.dram_tensor("lngv", [D], F32, kind="ExternalInput")
    lnbv = nc.dram_tensor("lnbv", [D], F32, kind="ExternalInput")
    maskc = nc.dram_tensor("maskc", [P, 4], F32, kind="ExternalInput")
    out = nc.dram_tensor("out", [4 * P, D], F32, kind="ExternalOutput")
    # exchange buffers (bf16): per 512-token chunk
    ein = [nc.dram_tensor(f"ein{j}", [4 * P, D], BF16) for j in range(QG)]
    eout = [nc.dram_tensor(f"eout{j}", [P, D], BF16) for j in range(QG)]

    def load_hilo_chunks(pool, name, src):
        """4 rotating [P, DCH, 2, 512] tiles (hi/lo interleaved)."""
        xs = []
        for n in range(S // 512):
            t = pool.tile([P, DCH, 2, 512], FP8, tag="xin",
                          name=f"{name}{n}")
            nc.sync.dma_start(t, src[:, :, :, n * 512:(n + 1) * 512])
            xs.append(t)
        return xs

    def emit_proj_qk(dst, wt_sb, xs, acc, bias_col, qscale):
        """dst: 2 x [P, S] tiles (feature-major).  12 DR matmuls per 512-col
        chunk: (w_hi,x_hi),(w_lo,x_hi),(w_hi,x_lo) over chunk pairs."""
        for nchk in range(S // 512):
            x_sb = xs[nchk]
            for ft in range(2):
                ps = acc.tile([P, 512], F32, tag="acc", name="acc")
                n = 0
                for c in range(0, DCH, 2):
                    for xj, wj in ((0, 0), (0, 1), (1, 0)):
                        mm(ps, wt_sb[:, c:c + 2, wj,
                                     ft * P:(ft + 1) * P],
                           x_sb[:, c:c + 2, xj, :],
                           start=(n == 0), stop=(n == 3 * DCH // 2 - 1),
                           perf_mode=DR)
                        n += 1
                if qscale is None:
                    nc.vector.tensor_scalar_add(
                        dst[ft][:, nchk * 512:(nchk + 1) * 512], ps,
                        bias_col[:, ft:ft + 1])
                else:
                    nc.vector.tensor_scalar(
                        dst[ft][:, nchk * 512:(nchk + 1) * 512], ps,
                        qscale, bias_col[:, ft:ft + 1], OP.mult, OP.add)

    def emit_proj_v(v_sb, wt_sb, xs, acc, bv_b, ones_t):
        for t in range(KC):
            x_sb = xs[t // 4]
            tc0 = (t % 4) * P
            ps = acc.tile([P, 512], F32, tag="acc", name="acc")
            n = 0
            for c in range(0, DCH, 2):
                for xj, wj in ((0, 0), (0, 1), (1, 0)):
                    mm(ps[:, 0:FTL],
                       x_sb[:, c:c + 2, xj, tc0:tc0 + P],
                       wt_sb[:, c:c + 2, wj, :],
                       start=(n == 0), stop=(n == 3 * DCH // 2 - 1),
                       perf_mode=DR)
                    n += 1
            nc.vector.tensor_copy(v_sb[t][:, :, DK:DK + 1], ones_t)
            nc.vector.tensor_tensor(
                v_sb[t][:, :, 0:DK],
                ps[:, 0:FTL].rearrange("p (h d) -> p h d", h=HL),
                bv_b[:].rearrange("p (h d) -> p h d", h=HL),
                OP.add)

    with tile.TileContext(nc) as tc:
        with (
            tc.tile_pool(name="const", bufs=1) as cp,
            tc.tile_pool(name="qTp", bufs=1) as qp,
            tc.tile_pool(name="kpp", bufs=1) as kpp,
            tc.tile_pool(name="vp", bufs=1) as vp,
        ):
            ident = cp.tile([P, P], F32, tag="ident", name="ident")
            make_identity(nc, ident)
            eps_t = cp.tile([P, 1], F32, tag="eps", name="eps")
            nc.vector.memset(eps_t, 1e-5)
            ones_t = cp.tile([P, HL, 1], F32, tag="ones", name="ones")
            nc.vector.memset(ones_t, WS)   # 64.0: cancels v's 64x scale
            bq_col = cp.tile([P, 2], F32, tag="bqc", name="bqc")
            bk_col = cp.tile([P, 2], F32, tag="bkc", name="bkc")
            b1_col = cp.tile([P, FKC], F32, tag="b1c", name="b1c")
            bv_b = cp.tile([P, FTL], F32, tag="bvb", name="bvb")
            mask_c = cp.tile([P, 4], F32, tag="mkc", name="mkc")
            # small consts go on the Activation DMA queue so they don't
            # delay the first projection input chunks on the SP queue
            nc.scalar.dma_start(bq_col, bqc[:])
            nc.scalar.dma_start(bk_col, bkc[:])
            nc.scalar.dma_start(b1_col, b1c[:])
            nc.scalar.dma_start(mask_c, maskc[:])
            nc.gpsimd.dma_start(bv_b, _bcast_ap(bvv[:]))
            b2_b = lng_b = lnb_b = None
            if not b2_zero:
                b2_b = cp.tile([P, D], F32, tag="b2b", name="b2b")
                nc.gpsimd.dma_start(b2_b, _bcast_ap(b2v[:]))
            if ln_affine:
                lng_b = cp.tile([P, D], F32, tag="lng", name="lng")
                lnb_b = cp.tile([P, D], F32, tag="lnb", name="lnb")
                nc.gpsimd.dma_start(lng_b, _bcast_ap(lngv[:]))
                nc.gpsimd.dma_start(lnb_b, _bcast_ap(lnbv[:]))

            qT = [qp.tile([P, S], BF16, tag=f"qT{m}", name=f"qT{m}")
                  for m in range(2)]
            kp = [kpp.tile([P, S], BF16, tag=f"kp{m}", name=f"kp{m}")
                  for m in range(2)]
            v_sb = [vp.tile([P, HL, DK + 1], BF16, tag=f"v{t}", name=f"v{t}")
                    for t in range(KC)]

            with tc.tile_pool(name="accp", bufs=2, space="PSUM") as acc:
                with (
                    tc.tile_pool(name="xin", bufs=10) as xip,
                    tc.tile_pool(name="wp", bufs=1) as wp,
                ):
                    wk_sb = wp.tile([P, DCH, 2, FTL], FP8, tag="wk",
                                    name="wk")
                    nc.sync.dma_start(wk_sb, wkt[:])
                    xs = load_hilo_chunks(xip, "xk", xk)
                    wq_sb = wp.tile([P, DCH, 2, FTL], FP8, tag="wq",
                                    name="wq")
                    nc.sync.dma_start(wq_sb, wqt[:])
                    emit_proj_qk(kp, wk_sb, xs, acc, bk_col, None)
                    xs = load_hilo_chunks(xip, "xq", xq)
                    wv_sb = wp.tile([P, DCH, 2, FTL], FP8, tag="wv",
                                    name="wv")
                    nc.sync.dma_start(wv_sb, wvt[:])
                    emit_proj_qk(qT, wq_sb, xs, acc, bq_col, float(qscale))
                    xs_v = load_hilo_chunks(xip, "xv", xv)
                    emit_proj_v(v_sb, wv_sb, xs_v, acc, bv_b, ones_t)

            from contextlib import ExitStack
            with ExitStack() as _es:
                pools = {}
                for nm, bufs, space in (
                    ("w1p", 1, None), ("w2p", 1, None), ("aE", 12, None),
                    ("aT", 2, None), ("aR", 2, None), ("attnp", 8, None),
                    ("stp", 4, None), ("ffinp", 2, None), ("ffip", 2, None),
                    ("ffiTp", 1, None), ("hTp", 1, None), ("outp", 2, None),
                    ("lnp", 4, None), ("t1p", 2, None),
                    ("psS", 2, "PSUM"), ("psA", 2, "PSUM"),
                    ("psH", 1, "PSUM"), ("psF", 1, "PSUM"),
                ):
                    kw = dict(name=nm, bufs=bufs)
                    if space:
                        kw["space"] = space
                    pools[nm] = _es.enter_context(tc.tile_pool(**kw))
                w1p, w2p, aE, aT, aR = (pools[k] for k in
                                        ("w1p", "w2p", "aE", "aT", "aR"))
                ap_, stp, fin_, fip, ftp = (pools[k] for k in
                                            ("attnp", "stp", "ffinp",
                                             "ffip", "ffiTp"))
                hp_, op_, lnp = pools["hTp"], pools["outp"], pools["lnp"]
                t1p = pools["t1p"]
                psS, psA, psH, psF = (pools[k] for k in
                                      ("psS", "psA", "psH", "psF"))
                w1_sb = w1p.tile([P, DCH, 2, FF], FP8, tag="w1", name="w1")
                nc.sync.dma_start(w1_sb, w1t[:])
                w2_sb = w2p.tile([P, FKC, 2, D], FP8, tag="w2", name="w2")
                nc.sync.dma_start(w2_sb, w2t[:])
                ffiT_h = ftp.tile([P, DCH, 512], FP8, tag="fTh", name="fTh")
                ffiT_l = ftp.tile([P, DCH, 512], FP8, tag="fTl", name="fTl")
                hT_h = hp_.tile([P, FKC, 512], FP8, tag="hTh", name="hTh")
                hT_l = hp_.tile([P, FKC, 512], FP8, tag="hTl", name="hTl")

                def emit_scores_half(qg, h, exps, half):
                    p, hp = h // 2, h % 2
                    lo, hi = hp * DK, (hp + 1) * DK
                    for g in range(half * 4, half * 4 + 4):
                        ps = psS.tile([P, 1024], F32, tag="psS", name="psS")
                        for j in range(2):
                            m = 2 * g + j
                            mm(ps[:, j * 512:(j + 1) * 512],
                               kp[p][lo:hi, m * P:(m + 1) * P],
                               qT[p][lo:hi, qg * 512:(qg + 1) * 512],
                               start=True, stop=True)
                        e = aE.tile([P, 1024], BF16, tag="exp", name="exp")
                        nc.scalar.activation(e, ps, AF.Exp)
                        exps.append(e)

                def emit_attnv(h, exps, attn_t):
                    pa = psA.tile([P, 512], F32, tag="pa", name="pa")
                    for m in range(KC):
                        mm(pa[:DK + 1], v_sb[m][:, h, :],
                           exps[m // 2][:, (m % 2) * 512:(m % 2 + 1) * 512],
                           start=(m == 0), stop=(m == KC - 1))
                    at = aT.tile([P, 512], F32, tag="at", name="at")
                    nc.vector.tensor_copy(at[:DK + 1], pa[:DK + 1])
                    rc = aR.tile([P, 4], F32, tag="rc", name="rc")
                    for qq in range(4):
                        pt = psA.tile([P, 512], F32, tag="pa", name="pa")
                        nc.tensor.transpose(
                            pt[:, :DK + 1], at[:DK + 1, qq * P:(qq + 1) * P],
                            ident[:DK + 1, :DK + 1])
                        nc.vector.reciprocal(rc[:, qq:qq + 1],
                                             pt[:, DK:DK + 1])
                        nc.vector.tensor_scalar_mul(
                            attn_t[qq][:, h * DK:(h + 1) * DK],
                            pt[:, 0:DK], rc[:, qq:qq + 1])

                def emit_staging_rs(qg, attn_t):
                    for qq in range(4):
                        st = stp.tile([P, D], BF16, tag="st", name="st")
                        for g in range(4):
                            nc.vector.tensor_scalar_mul(
                                st[:, g * FTL:(g + 1) * FTL], attn_t[qq],
                                mask_c[:, g:g + 1])
                        nc.sync.dma_start(
                            ein[qg][qq * P:(qq + 1) * P, :], st)
                    nc.gpsimd.collective_compute(
                        "ReduceScatter", OP.add, replica_groups=RG,
                        ins=[ein[qg][:]], outs=[eout[qg][:]])
                    pending.append(emit_ln_ffn_chunk(qg))

                def emit_attn_all(step):
                    """Flat head-stream across all q-groups: head (qg,h)'s
                    attnV runs between the score halves of the next head,
                    ALSO across qg boundaries, so the PE never waits for
                    the previous q-group's trailing exps."""
                    cur = {}

                    def attn_tiles(qg):
                        if qg not in cur:
                            cur[qg] = [ap_.tile([P, FTL], F32, tag="attn",
                                                name=f"at{qg}_{qq}")
                                       for qq in range(4)]
                        return cur[qg]

                    pend = None

                    def flush():
                        nonlocal pend
                        pqg, ph, pexps = pend
                        emit_attnv(ph, pexps, attn_tiles(pqg))
                        if ph == HL - 1:
                            emit_staging_rs(pqg, cur.pop(pqg))
                        pend = None

                    for qg in range(QG):
                        for h in range(HL):
                            exps = []
                            emit_scores_half(qg, h, exps, 0)
                            if pend is not None:
                                flush()
                            if STEP2:
                                step()
                            emit_scores_half(qg, h, exps, 1)
                            pend = (qg, h, exps)
                            step()
                    flush()

                def emit_ln_ffn_chunk(j):
                    """Generator: yields between pieces so LN/FFN
                    interleaves with the next q-group's attention."""
                    for _ in range(NOPS):
                        yield
                    ffin = fin_.tile([P, D], BF16, tag="ffin", name="ffin")
                    nc.sync.dma_start(ffin, eout[j][:])
                    stats = lnp.tile([P, 2, 6], F32, tag="st", name="st")
                    for sg in range(2):
                        nc.vector.bn_stats(stats[:, sg, :],
                                           ffin[:, sg * 512:(sg + 1) * 512])
                    mv = lnp.tile([P, 2], F32, tag="mv", name="mv")
                    nc.vector.bn_aggr(mv, stats)
                    # rstd = exp(-0.5*ln(var+eps)): Ln and Exp live in the
                    # same ACT table set as the attention Exp, so the LN
                    # path forces no act-table switches (Sqrt would).
                    lnv = lnp.tile([P, 1], F32, tag="sd", name="sd")
                    nc.scalar.activation(lnv, mv[:, 1:2], AF.Ln, bias=eps_t)
                    rstd = lnp.tile([P, 1], F32, tag="rs", name="rs")
                    nc.scalar.activation(rstd, lnv, AF.Exp, scale=-0.5)
                    fj = fip.tile([P, D], F32, tag="ffi", name=f"ffi{j}")
                    nc.vector.tensor_scalar(fj, ffin, mv[:, 0:1], rstd,
                                            OP.subtract, OP.mult)
                    if ln_affine:
                        nc.vector.tensor_mul(fj, fj, lng_b)
                        nc.vector.tensor_add(fj, fj, lnb_b)
                    # transpose to [D, tokens] and split hi/lo fp8.
                    # PSUM comes from the FFN2 pool, NOT psA: sharing psA
                    # would couple the attention attnV/transpose chain to
                    # this RS-gated work and stall the PE FIFO.
                    for cg in range(2):
                        pt = psF.tile([P, 512], F32, tag="psF", name="psF")
                        for c4 in range(4):
                            c8 = cg * 4 + c4
                            nc.tensor.transpose(
                                pt[:, c4 * P:(c4 + 1) * P],
                                fj[:, c8 * P:(c8 + 1) * P], ident)
                        hi_ap = ffiT_h[:, cg * 4:(cg + 1) * 4,
                                       j * P:(j + 1) * P]
                        lo_ap = ffiT_l[:, cg * 4:(cg + 1) * 4,
                                       j * P:(j + 1) * P]
                        nc.vector.tensor_copy(hi_ap, pt.rearrange(
                            "p (c t) -> p c t", c=4))
                        nc.vector.tensor_tensor(lo_ap, pt.rearrange(
                            "p (c t) -> p c t", c=4), hi_ap, OP.subtract)
                    yield
                    # FFN1: psH holds 4 fk regions of 128 tokens
                    for fg in range(4):
                        if fg and fg % FGY == 0:
                            yield
                        ps = psH.tile([P, 512], F32, tag="psH", name="psH")
                        for f4 in range(4):
                            fk = fg * 4 + f4
                            reg = ps[:, f4 * P:(f4 + 1) * P]
                            n = 0
                            for c in range(0, DCH, 2):
                                for x_t, wj in ((ffiT_h, 0), (ffiT_h, 1),
                                                (ffiT_l, 0)):
                                    mm(reg,
                                       w1_sb[:, c:c + 2, wj,
                                             fk * P:(fk + 1) * P],
                                       x_t[:, c:c + 2, j * P:(j + 1) * P],
                                       start=(n == 0),
                                       stop=(n == 3 * DCH // 2 - 1),
                                       perf_mode=DR)
                                    n += 1
                        t1 = t1p.tile([P, 512], F32R, tag="t1", name="t1")
                        if b1_zero:
                            nc.scalar.activation(t1, ps, AF.Relu)
                            hi_ap = hT_h[:, fg * 4:(fg + 1) * 4,
                                         j * P:(j + 1) * P]
                            lo_ap = hT_l[:, fg * 4:(fg + 1) * 4,
                                         j * P:(j + 1) * P]
                            t1r = t1.rearrange("p (c t) -> p c t", c=4)
                            nc.vector.tensor_copy(hi_ap, t1r)
                            nc.vector.tensor_tensor(lo_ap, t1r, hi_ap,
                                                    OP.subtract)
                        else:
                            for f4 in range(4):
                                fk = fg * 4 + f4
                                sl = slice(f4 * P, (f4 + 1) * P)
                                nc.vector.tensor_scalar(
                                    t1[:, sl], ps[:, sl],
                                    b1_col[:, fk:fk + 1], 0.0,
                                    OP.add, OP.max)
                            t1r = t1.rearrange("p (c t) -> p c t", c=4)
                            hi_ap = hT_h[:, fg * 4:(fg + 1) * 4,
                                         j * P:(j + 1) * P]
                            lo_ap = hT_l[:, fg * 4:(fg + 1) * 4,
                                         j * P:(j + 1) * P]
                            nc.vector.tensor_copy(hi_ap, t1r)
                            nc.vector.tensor_tensor(lo_ap, t1r, hi_ap,
                                                    OP.subtract)
                    yield
                    # FFN2 + residual
                    o_sb = op_.tile([P, D], F32, tag="out", name="out")
                    for half in range(2):
                        if half == 1:
                            yield
                        ps = psF.tile([P, 512], F32, tag="psF", name="psF")
                        n = 0
                        for fk in range(0, FKC, 2):
                            for x_t, wj in ((hT_h, 0), (hT_h, 1),
                                            (hT_l, 0)):
                                mm(ps,
                                   x_t[:, fk:fk + 2, j * P:(j + 1) * P],
                                   w2_sb[:, fk:fk + 2, wj,
                                         half * 512:(half + 1) * 512],
                                   start=(n == 0),
                                   stop=(n == 3 * FKC // 2 - 1),
                                   perf_mode=DR)
                                n += 1
                        sl = slice(half * 512, (half + 1) * 512)
                        nc.vector.tensor_scalar_mul(
                            o_sb[:, sl], ps, 1.0 / (WS * WS))
                        nc.vector.tensor_add(o_sb[:, sl], o_sb[:, sl],
                                             fj[:, sl])
                        if not b2_zero:
                            nc.vector.tensor_add(o_sb[:, sl], o_sb[:, sl],
                                                 b2_b[:, sl])
                        nc.sync.dma_start(out[j * P:(j + 1) * P, sl],
                                          o_sb[:, sl])

                # software pipeline: LN/FFN pieces of chunk j are
                # drained at points inside attention of later q-groups so
                # the ReduceScatter latency never stalls the PE FIFO
                from collections import deque
                pending = deque()

                def step():
                    while pending:
                        try:
                            next(pending[0])
                            return
                        except StopIteration:
                            pending.popleft()

                def drain_front():
                    # exhaust the current front generator (used to force
                    # V-projection completion before the first attnV)
                    if not pending:
                        return
                    g = pending[0]
                    while True:
                        try:
                            next(g)
                        except StopIteration:
                            pending.popleft()
                            return

                emit_attn_all(step)
                while pending:
                    step()

    nc.compile()
    return nc


def _hilo(a):
    import ml_dtypes
    hi = a.astype(ml_dtypes.float8_e4m3)
    lo = (a - hi.astype(np.float32)).astype(ml_dtypes.float8_e4m3)
    return hi, lo


def _chunk_major(a):
    """[D, S] -> [P, DCH, S] with row k = c*128 + p."""
    return np.ascontiguousarray(a.reshape(DCH, P, -1).transpose(1, 0, 2))


def _w_ilv(w):
    """[D, M] scaled weights -> hi/lo interleaved [P, DCH, 2, M] fp8."""
    hi, lo = _hilo(w)
    M = w.shape[1]
    st = np.stack([hi.reshape(DCH, P, M), lo.reshape(DCH, P, M)], axis=2)
    return np.ascontiguousarray(st.transpose(1, 0, 2, 3))


def _w2_ilv(w):
    """[FF, D] scaled weights -> hi/lo interleaved [P, FKC, 2, D] fp8."""
    hi, lo = _hilo(w)
    st = np.stack([hi.reshape(FKC, P, D), lo.reshape(FKC, P, D)], axis=2)
    return np.ascontiguousarray(st.transpose(1, 0, 2, 3))


def kernel(**inputs) -> np.ndarray:
    f32 = lambda a: np.asarray(a, dtype=np.float32)
    query, key, value = f32(inputs["query"]), f32(inputs["key"]), f32(inputs["value"])
    s = 1.0 / np.sqrt(np.float32(DK))
    Wq, Wk, Wv = f32(inputs["Wq"]), f32(inputs["Wk"]), f32(inputs["Wv"])
    bq, bk, bv = f32(inputs["bq"]), f32(inputs["bk"]), f32(inputs["bv"])
    W1, W2 = f32(inputs["W1"]), f32(inputs["W2"])
    b1, b2 = f32(inputs["b1"]), f32(inputs["b2"])
    ln_g, ln_b = f32(inputs["ln_g"]), f32(inputs["ln_b"])

    ln_affine = not (np.all(ln_g == 1.0) and np.all(ln_b == 0.0))
    nc = build_program(ln_affine=ln_affine, b1_zero=not b1.any(),
                       b2_zero=not b2.any())

    w1i = _w_ilv(W1 * WS)
    w2i = _w2_ilv(W2 * WS)
    b1c = np.ascontiguousarray(b1.reshape(FKC, P).T) * WS
    shared = dict(w1t=w1i, w2t=w2i, b1c=b1c, b2v=b2, lngv=ln_g, lnbv=ln_b)

    in_maps = []
    for c in range(N_CORES):
        b = c // 4
        r = c % 4
        cols = slice(r * FTL, (r + 1) * FTL)
        mask = np.zeros((P, 4), np.float32)
        mask[:, r] = 1.0
        in_maps.append(dict(
            xq=_w_ilv(query[b].T),
            xk=_w_ilv(key[b].T),
            xv=_w_ilv(value[b].T),
            wqt=_w_ilv(Wq[:, cols] * WS),
            wkt=_w_ilv(Wk[:, cols] * WS),
            wvt=_w_ilv(Wv[:, cols] * WS),
            bqc=np.ascontiguousarray(bq[cols].reshape(2, P).T) * (s / WS),
            bkc=np.ascontiguousarray(bk[cols].reshape(2, P).T) * WS,
            bvv=bv[cols] * WS,
            maskc=mask,
            **shared,
        ))

    res = run_bass_kernel_spmd(nc, in_maps, list(range(N_CORES)))
    out = np.empty((B, S, D), dtype=np.float32)
    for c in range(N_CORES):
        b = c // 4
        r = c % 4
        o = res.results[c]["out"]  # [512, D]
        for j in range(QG):
            t0 = j * 512 + r * P
            out[b, t0:t0 + P, :] = o[j * P:(j + 1) * P, :]
    return out
